# revision 14
# baseline (speedup 1.0000x reference)
"""Trainium2 Bass kernel for CTUNOBlock1D (spectral conv + time conv + batchnorm + relu).

Strategy (data-parallel over batch, 8 cores, 4 batches/core):
  - rfft uses only 33 modes -> DFT as matmuls against trig tables. x is
    shipped per l-chunk as [x0|x1|cst|x2|x3] so ONE matmul per batch-pair
    per chunk computes both the DFT (X = x^T cst) and the Gram blocks
    (x^T x, for exact BN stats via Parseval) with a 128-wide stationary.
  - mode mixing packs [Wr_k|Wi_k] into one 128-col stationary -> 33 matmuls.
  - BN stats are computed in mode space and AllReduced (64x2 f32) as early
    as possible; the collective overlaps the inverse/residual phase.
  - residual branch folded on host: E_b = K diag(w_t) Wt^T; device computes
    out^T = [Z;E]^T @ [ABt; x^T] per batch (channel-major, K=128 concat).
    PSUM drains are split across Scalar/Vector/Pool engines.
  - BN apply + ReLU is split across Scalar/Vector/Pool with per-chunk
    streaming DMA writes; output is transposed on host.
"""

import os
import numpy as np

import concourse.bass as bass
import concourse.mybir as mybir
import concourse.bacc as bacc
import concourse.tile as tile
from concourse import bass_utils

F32 = mybir.dt.float32
BF16 = mybir.dt.bfloat16
NP_BF16 = mybir.dt.np(BF16)

B, L, CIN, COUT, TEMB = 32, 8192, 64, 64, 256
M = 33            # retained rfft modes
KC = 2 * M        # 66 (real|imag concat)
NCORES = 8
BLOC = B // NCORES   # 4 batches per core
EPS = 1e-5
NCHUNK = L // 128    # 64 l-chunks of 128
CW = 322             # per-chunk xq cols: x0|x1|cst|x2|x3 = 64+64+66+64+64
USE_AR = bool(int(os.environ.get("KBENCH_AR", "1")))
USE_AG = bool(int(os.environ.get("KBENCH_AG", "0")))
USE_RD = bool(int(os.environ.get("KBENCH_RD", "0")))
WARM_CC = bool(int(os.environ.get("KBENCH_WARMCC", "0")))
NXPIECE = 8          # xq DMA split


def _build():
    nc = bacc.Bacc(None, target_bir_lowering=False)

    xq_d = nc.dram_tensor("xq", [128, NCHUNK * CW], BF16, kind="ExternalInput")
    xt_d = nc.dram_tensor("xt", [BLOC, CIN, L], BF16, kind="ExternalInput")
    abt_d = nc.dram_tensor("abt", [CIN, L], BF16, kind="ExternalInput")
    wm_d = nc.dram_tensor("wm", [CIN, M * 128], BF16, kind="ExternalInput")
    ebf_d = nc.dram_tensor("ebf", [CIN, BLOC * COUT], BF16, kind="ExternalInput")
    ef_d = nc.dram_tensor("ef", [CIN, BLOC * COUT], F32, kind="ExternalInput")
    tm_d = nc.dram_tensor("tm", [COUT, 2 * 4 * M], F32, kind="ExternalInput")
    e4_d = nc.dram_tensor("e4", [COUT, BLOC], F32, kind="ExternalInput")
    ep_d = nc.dram_tensor("ep", [128, 2], F32, kind="ExternalInput")
    bnp_d = nc.dram_tensor("bnp", [128, 2], F32, kind="ExternalInput")
    id_d = nc.dram_tensor("idm", [64, 64], F32, kind="ExternalInput")
    out_d = nc.dram_tensor("out", [BLOC, COUT, L], BF16, kind="ExternalOutput")

    rd_ref = {}
    with tile.TileContext(nc) as tc:
        with (
            tc.tile_pool(name="const", bufs=1) as cpool,
            tc.tile_pool(name="xs", bufs=1) as xpool,
            tc.tile_pool(name="xtp", bufs=1) as xtpool,
            tc.tile_pool(name="outb", bufs=1) as opool,
            tc.tile_pool(name="small", bufs=2) as spool,
            tc.tile_pool(name="psA", bufs=1, space=bass.MemorySpace.PSUM) as psA,
            tc.tile_pool(name="psS", bufs=2, space=bass.MemorySpace.PSUM) as psS,
            tc.tile_pool(name="psB", bufs=3, space=bass.MemorySpace.PSUM) as psB,
            tc.tile_pool(name="dram", bufs=1, space=bass.MemorySpace.DRAM) as dpool,
        ):
            dma = nc.sync.dma_start
            TT = nc.vector.tensor_tensor
            TS = nc.vector.tensor_scalar
            gTT = nc.gpsimd.tensor_tensor
            gTS = nc.gpsimd.tensor_scalar
            OP = mybir.AluOpType

            xq = xpool.tile([128, NCHUNK * CW], BF16, tag="xq")
            PW = NCHUNK // NXPIECE * CW
            for p in range(NXPIECE):
                dma(xq[:, PW * p:PW * (p + 1)], xq_d[:, PW * p:PW * (p + 1)])

            # small constants (own ring position; cheap)
            wm_s = cpool.tile([CIN, M * 128], BF16)
            tm_s = cpool.tile([COUT, 2 * 4 * M], F32)
            e4_s = cpool.tile([COUT, BLOC], F32)
            ep_s = cpool.tile([128, 2], F32)
            bnp_s = cpool.tile([128, 2], F32)
            id_s = cpool.tile([64, 64], F32)
            ones_s = cpool.tile([64, 1], F32)
            dma(wm_s[:], wm_d[:])
            dma(tm_s[:], tm_d[:])
            dma(e4_s[:], e4_d[:])
            dma(ep_s[:], ep_d[:])
            dma(bnp_s[:], bnp_d[:])
            dma(id_s[:], id_d[:])
            nc.vector.memset(ones_s[:], 1.0)

            ebf_s = cpool.tile([CIN, BLOC * COUT], BF16)   # [i, 64b+o]
            ef_s = cpool.tile([CIN, BLOC * COUT], F32)
            zeb = cpool.tile([128, BLOC * COUT], BF16)     # [0:64]=Z^T, [64:128]=E
            dma(ebf_s[:], ebf_d[:])
            dma(ef_s[:], ef_d[:])
            dma(zeb[64:128, :], ebf_d[:])

            # early dummy collective: absorbs the cross-core launch barrier
            # and warms the CC rings while compute proceeds
            if WARM_CC and (USE_AR or USE_AG) and not USE_RD:
                wtin = dpool.tile([2, 2], F32)
                wtout = dpool.tile([2, 2], F32)
                wsrc = spool.tile([2, 2], F32, tag="wsrc")
                nc.vector.memset(wsrc[:], 0.0)
                nc.gpsimd.dma_start(wtin[:], wsrc[:])
                nc.gpsimd.collective_compute(
                    "AllReduce", mybir.AluOpType.add,
                    replica_groups=[list(range(NCORES))],
                    ins=[wtin.opt()], outs=[wtout.opt()],
                )

            # early dummy Sqrt to pre-load the ACT table set, and a dummy
            # gpsimd op to pre-load the Pool ext-isa library
            warm = spool.tile([1, 1], F32)
            nc.vector.memset(warm[:], 1.0)
            nc.scalar.activation(warm[:], warm[:], mybir.ActivationFunctionType.Sqrt)
            warm2 = spool.tile([1, 1], F32, tag="warm2")
            nc.gpsimd.tensor_tensor(warm2[:], warm[:], warm[:], mybir.AluOpType.add)

            # bulk phase-C inputs, gated behind the critical xq loads: the
            # dummy read of the last xq piece makes the sync ring wait before
            # issuing these transfers (keeps HBM bandwidth on the fwd path).
            gate = spool.tile([1, 2], BF16, tag="gate")
            dma(gate[:], xq[0:1, NCHUNK * CW - 2:NCHUNK * CW])
            xtc = []
            for b in range(BLOC):
                xt = xtpool.tile([128, L], BF16, tag=f"xtc{b}")
                xtc.append(xt)
            dma(xtc[0][0:64, :], abt_d[:])
            for b in range(BLOC):
                dma(xtc[b][64:128, :], xt_d[b])
            # replicate the ABt table into the other xtc tops off the sync ring
            nc.scalar.dma_start(xtc[1][0:64, :], xtc[0][0:64, :])
            nc.scalar.dma_start(xtc[2][0:64, :], xtc[0][0:64, :])
            nc.scalar.dma_start(xtc[3][0:64, :], xtc[1][0:64, :])

            # ---- phase A: one matmul per batch-pair per chunk ----
            # pair0: [x0|x1]^T @ [x0|x1|cst] -> [G00 G01; G10 G11 | X0; X1]
            # pair1: [x2|x3]^T @ [cst|x2|x3] -> [X2; X3 | G22 .. ; .. G33]
            pA0 = psA.tile([128, 194], F32, tag="pA0")
            pA1 = psA.tile([128, 194], F32, tag="pA1")
            for u in range(NCHUNK):
                o = CW * u
                nc.tensor.matmul(pA0[:], xq[:, o:o + 128], xq[:, o:o + 194],
                                 start=(u == 0), stop=(u == NCHUNK - 1))
                nc.tensor.matmul(pA1[:], xq[:, o + 194:o + 322], xq[:, o + 128:o + 322],
                                 start=(u == 0), stop=(u == NCHUNK - 1))

            # ---- copies out of phase-A PSUM (split across engines) ----
            Xsb = cpool.tile([CIN, BLOC * KC], BF16)    # [c, 66b+(ri,k)]
            Gsb = cpool.tile([CIN, BLOC * CIN], BF16)   # [c, 64b+c']
            nc.scalar.copy(Xsb[:, 0:66], pA0[0:64, 128:194])
            nc.vector.tensor_copy(Xsb[:, 66:132], pA0[64:128, 128:194])
            nc.scalar.copy(Xsb[:, 132:198], pA1[0:64, 0:66])
            nc.scalar.copy(Xsb[:, 198:264], pA1[64:128, 0:66])
            nc.vector.tensor_copy(Gsb[:, 0:64], pA0[0:64, 0:64])
            nc.vector.tensor_copy(Gsb[:, 64:128], pA0[64:128, 64:128])
            nc.scalar.copy(Gsb[:, 128:192], pA1[0:64, 66:130])
            nc.vector.tensor_copy(Gsb[:, 192:256], pA1[64:128, 130:194])

            # ---- phase B: mode mixing, one matmul per mode ----
            # P[0:64,8k+(j,b)] = Wr_k^T @ [Xr|Xi]; P[64:128,...] = Wi_k^T @ ...
            Pp = psS.tile([128, M * 8], F32, tag="ps_s")
            Xr4 = Xsb[:].rearrange("p (b j k) -> p j b k", b=BLOC, j=2, k=M)
            for k in range(M):
                nc.tensor.matmul(Pp[:, 8 * k:8 * (k + 1)],
                                 wm_s[:, 128 * k:128 * (k + 1)],
                                 Xr4[:, :, :, k], start=True, stop=True)

            # Gm / m1p matmuls (independent of P; share the PE queue)
            gmp = psS.tile([COUT, BLOC * KC], F32, tag="ps_s")
            m1p = psS.tile([CIN, BLOC * COUT], F32, tag="ps_s")
            for b in range(BLOC):
                nc.tensor.matmul(gmp[:, KC * b:KC * (b + 1)],
                                 ebf_s[:, 64 * b:64 * (b + 1)],
                                 Xsb[:, KC * b:KC * (b + 1)], start=True, stop=True)
                nc.tensor.matmul(m1p[:, 64 * b:64 * (b + 1)],
                                 Gsb[:, 64 * b:64 * (b + 1)],
                                 ebf_s[:, 64 * b:64 * (b + 1)], start=True, stop=True)

            Psb = spool.tile([COUT, 2 * M * 8], F32, tag="psb")
            nc.scalar.copy(Psb[:, 0:M * 8], Pp[0:64, :])
            nc.vector.tensor_copy(Psb[:, M * 8:2 * M * 8], Pp[64:128, :])

            Gm = cpool.tile([COUT, BLOC * KC], F32)     # [o, 66b+33ri+k]
            nc.scalar.copy(Gm[:], gmp[:])
            em = spool.tile([CIN, BLOC * COUT], F32, tag="em")
            TT(em[:], m1p[:], ef_s[:], OP.mult)
            qp = psS.tile([COUT, BLOC], F32, tag="ps_s")
            for b in range(BLOC):
                nc.tensor.matmul(qp[:, b:b + 1], em[:, 64 * b:64 * (b + 1)],
                                 ones_s[:], start=True, stop=True)

            Yr = spool.tile([COUT, 4 * M], F32, tag="yr")   # [(k,b)] = 4k+b
            Yi = spool.tile([COUT, 4 * M], F32, tag="yi")
            Pk1 = Psb[:, 0:M * 8].rearrange("p (k x) -> p k x", k=M, x=8)
            Pk2 = Psb[:, M * 8:2 * M * 8].rearrange("p (k x) -> p k x", k=M, x=8)
            Yrv = Yr[:].rearrange("p (k b) -> p k b", k=M, b=4)
            Yiv = Yi[:].rearrange("p (k b) -> p k b", k=M, b=4)
            TT(Yrv, Pk1[:, :, 0:4], Pk2[:, :, 4:8], OP.subtract)
            gTT(Yiv, Pk2[:, :, 0:4], Pk1[:, :, 4:8], OP.add)
            # Yr used by DVE (t1,t4), Yi by Pool (t2,t3) -- minimal cross-hops

            Zsb = cpool.tile([COUT, 2 * 4 * M], F32)  # [(ri,k,b)] = 132ri+4k+b
            t1 = spool.tile([COUT, 4 * M], F32, tag="t1")
            t2 = spool.tile([COUT, 4 * M], F32, tag="t2")
            t3 = spool.tile([COUT, 4 * M], F32, tag="t3")
            t4 = spool.tile([COUT, 4 * M], F32, tag="t4")
            ntm = 4 * M
            TT(t1[:], Yr[:], tm_s[:, 0:ntm], OP.mult)
            TT(t4[:], Yr[:], tm_s[:, ntm:2 * ntm], OP.mult)
            gTT(t2[:], Yi[:], tm_s[:, ntm:2 * ntm], OP.mult)
            gTT(t3[:], Yi[:], tm_s[:, 0:ntm], OP.mult)
            TT(Zsb[:, 0:ntm], t1[:], t2[:], OP.subtract)
            TT(Zsb[:, ntm:2 * ntm], t3[:], t4[:], OP.add)

            # ---- stats in mode space (batched over the 4 batches) ----
            q4 = spool.tile([COUT, BLOC], F32, tag="q4")
            A12 = spool.tile([COUT, BLOC], F32, tag="a12")
            nc.vector.tensor_copy(q4[:], qp[:])
            # A12 = sum_k>=1 Zr*(Zr+2Gr) + Zi*(Zi+2Gi)  (= A1 + 2*A2)
            Zall = Zsb[:].rearrange("p (ri k b) -> p b ri k", ri=2, k=M, b=4)[:, :, :, 1:M]
            Gall = Gm[:].rearrange("p (b ri k) -> p b ri k", b=BLOC, ri=2, k=M)[:, :, :, 1:M]
            w256a = spool.tile([COUT, BLOC * 64], F32, tag="w256a")
            w256b = spool.tile([COUT, BLOC * 64], F32, tag="w256b")
            wa = w256a[:].rearrange("p (b ri k) -> p b ri k", b=4, ri=2, k=M - 1)
            wb = w256b[:].rearrange("p (b ri k) -> p b ri k", b=4, ri=2, k=M - 1)
            gTS(wa, Gall, 2.0, 0.0, OP.mult, OP.add)
            TT(wb, Zall, wa, OP.add)
            TT(wa, Zall, wb, OP.mult)
            nc.vector.tensor_reduce(
                A12[:], w256a[:].rearrange("p (b k) -> p b k", b=BLOC, k=64),
                mybir.AxisListType.X, OP.add)

            # vectorized S1/S2 assembly over the 4 batches
            Zr04 = Zsb[:, 0:4]                                  # Zr[k=0] per b
            u4 = Gm[:].rearrange("p (b x) -> p b x", b=BLOC, x=KC)[:, :, 0]
            v4 = spool.tile([COUT, BLOC], F32, tag="v4")
            s2c = spool.tile([COUT, BLOC], F32, tag="s2c")
            w1 = spool.tile([COUT, BLOC], F32, tag="w1")
            w2 = spool.tile([COUT, BLOC], F32, tag="w2")
            TT(v4[:], Zr04, u4, OP.add)
            TT(v4[:], v4[:], e4_s[:], OP.add)                   # v = Zr0+u+e
            gTT(w2[:], Zr04, u4, OP.mult)                       # Zr0*u
            TT(s2c[:], Zr04, Zr04, OP.mult)                     # Zr0^2
            TS(w1[:], A12[:], 2.0, 0.0, OP.mult, OP.add)
            TT(s2c[:], s2c[:], w1[:], OP.add)
            TS(w1[:], q4[:], 1.0 / L, 0.0, OP.mult, OP.add)
            TT(s2c[:], s2c[:], w1[:], OP.add)
            TT(w1[:], e4_s[:], v4[:], OP.mult)
            TT(w1[:], w1[:], w2[:], OP.add)                     # e*v + Zr0*u
            TS(w1[:], w1[:], 2.0, 0.0, OP.mult, OP.add)
            TT(s2c[:], s2c[:], w1[:], OP.add)
            TT(w1[:], e4_s[:], e4_s[:], OP.mult)
            TT(s2c[:], s2c[:], w1[:], OP.subtract)

            stat_in = spool.tile([COUT, 2], F32, tag="stin")
            nc.vector.tensor_reduce(stat_in[:, 0:1], v4[:], mybir.AxisListType.X, OP.add)
            nc.vector.tensor_reduce(stat_in[:, 1:2], s2c[:], mybir.AxisListType.X, OP.add)

            # ---- cross-core reduction of (64,2) stats ----
            st128 = spool.tile([128, 2], F32, tag="st128")
            if USE_RD:
                # P2P stats exchange over remote SBUF-to-SBUF DMA: each core
                # broadcasts its 512B stats to the 7 peers (slot j written by
                # peer me^j), then reduces locally once 7x2 sem bumps land.
                rsem = nc.alloc_semaphore(name="rd_rsem")
                lsem = nc.alloc_semaphore(name="rd_lsem")
                st_loc = cpool.tile([128, 1], F32)   # v on 0:64, s2 on 64:128
                nc.scalar.copy(st_loc[0:64, :], stat_in[:, 0:1])
                nc.scalar.copy(st_loc[64:128, :], stat_in[:, 1:2])
                rbuf = cpool.tile([128, 8], F32)
                nc.vector.tensor_copy(rbuf[:, 0:1], st_loc[:])   # self slot 0
                for dt in range(1, NCORES):
                    rdests = [None] * 8
                    rdests[dt] = (0, dt)
                    nc.gpsimd.remote_dma_broadcast(
                        rbuf[:, dt:dt + 1], st_loc[:], rsem, lsem, rdests=rdests)
                nc.gpsimd.trigger_dma(count=None)
                red = spool.tile([128, 1], F32, tag="red")
                ri = nc.vector.tensor_reduce(red[:], rbuf[:],
                                             mybir.AxisListType.X, OP.add)
                # the remote-sem wait is attached AFTER Tile scheduling (the
                # scheduling sim cannot model peer increments and deadlocks)
                rd_ref["ri"] = ri
                rd_ref["rsem"] = rsem
                nc.scalar.copy(st128[0:64, 0:1], red[0:64, :])
                nc.scalar.copy(st128[64:128, 0:1], red[0:64, :])
                nc.scalar.copy(st128[0:64, 1:2], red[64:128, :])
                nc.scalar.copy(st128[64:128, 1:2], red[64:128, :])
            elif USE_AG:
                din = dpool.tile([COUT, 2], F32)
                dout = dpool.tile([NCORES, COUT * 2], F32)
                nc.gpsimd.dma_start(din[:], stat_in[:])
                nc.gpsimd.collective_compute(
                    "AllGather", OP.bypass,
                    replica_groups=[list(range(NCORES))],
                    ins=[din.opt()], outs=[dout.opt()],
                )
                allst = spool.tile([COUT, NCORES * 2], F32, tag="allst")
                nc.scalar.dma_start(
                    allst[:].rearrange("p (r s) -> p r s", r=NCORES, s=2),
                    dout[:].rearrange("r (p s) -> p r s", p=COUT, s=2))
                st64 = spool.tile([COUT, 2], F32, tag="st64")
                av = allst[:].rearrange("p (r s) -> p s r", r=NCORES, s=2)
                nc.vector.tensor_reduce(st64[:], av, mybir.AxisListType.X, OP.add)
                nc.scalar.dma_start(st128[0:64, :], st64[:])
                nc.scalar.dma_start(st128[64:128, :], st64[:])
            else:
                din = dpool.tile([COUT, 2], F32)
                dout = dpool.tile([COUT, 2], F32)
                nc.gpsimd.dma_start(din[:], stat_in[:])
                if USE_AR:
                    nc.gpsimd.collective_compute(
                        "AllReduce", OP.add,
                        replica_groups=[list(range(NCORES))],
                        ins=[din.opt()], outs=[dout.opt()],
                    )
                else:
                    nc.gpsimd.dma_start(dout[:], din[:])
                nc.scalar.dma_start(st128[0:64, :], dout[:])
                nc.scalar.dma_start(st128[64:128, :], dout[:])

            # ---- Z transpose (per batch, DC row folded into bias later) ----
            # zeb[b]: rows 0:64 = Z^T (modes k=1..32, re|im), rows 64:128 = E_b
            Zflat = spool.tile([COUT, 4 * 64], F32, tag="zflat")  # [b][ri,k>=1]
            nc.vector.tensor_copy(
                Zflat[:].rearrange("p (b ri k) -> p b ri k", b=4, ri=2, k=M - 1),
                Zsb[:].rearrange("p (ri k b) -> p b ri k", ri=2, k=M, b=4)[:, :, :, 1:M])
            for b in range(BLOC):
                tp = psA.tile([CIN, COUT], F32, tag="ps_tp")
                nc.tensor.transpose(tp[:], Zflat[:, 64 * b:64 * (b + 1)], id_s[:])
                nc.vector.tensor_copy(zeb[0:64, 64 * b:64 * (b + 1)], tp[:])

            # Zr0 (DC) pair-stacking via partition-shifted engine copies
            zr0p = spool.tile([128, 2], F32, tag="zr0p")
            zr0v = zr0p[:].rearrange("p (j a) -> p j a", j=2, a=1)
            nc.scalar.copy(zr0v[0:64, :, 0],
                           Zsb[:, 0:4].rearrange("p (k b) -> p b k", k=1, b=4)[:, 0:4:2, 0])
            nc.scalar.copy(zr0v[64:128, :, 0],
                           Zsb[:, 0:4].rearrange("p (k b) -> p b k", k=1, b=4)[:, 1:4:2, 0])

            # ---- phase C: single K=128 matmul per tile: [Z;E]^T @ [ABt;xT] ----
            OUT = []
            for j in range(2):
                outj = opool.tile([128, L], BF16, tag=f"out{j}")
                OUT.append(outj)

            NSTEP = 512
            drains = [nc.vector.tensor_copy, nc.scalar.copy]
            di = 0
            for j in range(2):
                b0, b1 = 2 * j, 2 * j + 1
                for n in range(L // NSTEP):
                    ps = psB.tile([128, NSTEP], F32, tag="invres")
                    sl = slice(NSTEP * n, NSTEP * (n + 1))
                    nc.tensor.matmul(ps[0:64, :], zeb[:, 64 * b0:64 * b0 + 64],
                                     xtc[b0][:, sl], start=True, stop=True)
                    nc.tensor.matmul(ps[64:128, :], zeb[:, 64 * b1:64 * b1 + 64],
                                     xtc[b1][:, sl], start=True, stop=True,
                                     tile_position=(0, 64))
                    drains[di % 2](OUT[j][:, sl], ps[:])
                    di += 1

            # ---- BN scale/shift from all-reduced stats ----
            mean = spool.tile([128, 1], F32, tag="mean")
            ex2 = spool.tile([128, 1], F32, tag="ex2")
            var = spool.tile([128, 1], F32, tag="var")
            sv = spool.tile([128, 1], F32, tag="sv")
            sh = spool.tile([128, 1], F32, tag="sh")
            wk = spool.tile([128, 1], F32, tag="wk")
            TS(mean[:], st128[:, 0:1], 1.0 / B, 0.0, OP.mult, OP.add)
            TS(ex2[:], st128[:, 1:2], 1.0 / B, 0.0, OP.mult, OP.add)
            TT(wk[:], mean[:], mean[:], OP.mult)
            TT(var[:], ex2[:], wk[:], OP.subtract)
            TS(var[:], var[:], 1.0, EPS, OP.mult, OP.add)
            nc.scalar.activation(wk[:], var[:], mybir.ActivationFunctionType.Sqrt)
            nc.vector.reciprocal(sv[:], wk[:])
            TT(sv[:], sv[:], bnp_s[:, 0:1], OP.mult)            # s = bn_scale/std
            TT(wk[:], mean[:], sv[:], OP.mult)
            TT(sh[:], bnp_s[:, 1:2], wk[:], OP.subtract)        # shift = bias - mean*s

            bjs = []
            for j in range(2):
                bj = spool.tile([128, 1], F32, tag=f"bj{j}")
                TT(bj[:], ep_s[:, j:j + 1], zr0p[:, j:j + 1], OP.add)
                TT(bj[:], bj[:], sv[:], OP.mult)                # s*(e_b + Zr0)
                TT(bj[:], bj[:], sh[:], OP.add)                 # + shift
                bjs.append(bj)

            # ---- apply BN+ReLU split over ACT(1-pass) / DVE(2-pass, 2x bf16),
            # streaming each applied chunk out on alternating DMA rings ----
            NQ = 2048
            plan = {(0, 0): 'a', (0, 1): 'v', (1, 0): 'v', (1, 1): 'a',
                    (2, 0): 'a', (2, 1): 'v', (3, 0): 'v', (3, 1): 'v'}
            wrings = [nc.sync.dma_start, nc.scalar.dma_start]
            for n2 in range(4):
                for j in range(2):
                    q = slice(n2 * NQ, (n2 + 1) * NQ)
                    if plan[(n2, j)] == 'a':
                        nc.scalar.activation(OUT[j][:, q], OUT[j][:, q],
                                             mybir.ActivationFunctionType.Relu,
                                             bias=bjs[j][:], scale=sv[:])
                    else:
                        TS(OUT[j][:, q], OUT[j][:, q], sv[:], bjs[j][:],
                           OP.mult, OP.add)
                        TS(OUT[j][:, q], OUT[j][:, q], 0.0, 0.0, OP.max, OP.add)
                    od = out_d[2 * j:2 * j + 2].rearrange("a b l -> (a b) l")
                    wrings[(2 * n2 + j) % 2](od[:, q], OUT[j][:, q])

    if rd_ref:
        rd_ref["ri"].wait_op(rd_ref["rsem"], 2 * (NCORES - 1), "sem-ge", check=False)
    nc.compile()
    return nc


_NC_CACHE = {}


def _get_nc():
    if "nc" not in _NC_CACHE:
        _NC_CACHE["nc"] = _build()
    return _NC_CACHE["nc"]


def _host_prep(x, t_emb, spec_w_real, spec_w_imag, dense_re, dense_im,
               conv_kernel, conv_bias, tc_weights, psi_kernel, bn_scale, bn_bias):
    """Build per-core input maps (small tensors precomputed on host)."""
    k = np.arange(M)
    l = np.arange(L)
    ang = 2.0 * np.pi * np.outer(l, k) / L
    CSt = np.concatenate([np.cos(ang) / L, -np.sin(ang) / L], axis=1)   # (L, 66)
    angk = ang[:, 1:]                                # drop DC mode
    ABt = np.concatenate([(2.0 * np.cos(angk)).T,
                          (-2.0 * np.sin(angk)).T], axis=0).astype(np.float32)

    tr = (t_emb @ dense_re).astype(np.float32)      # (B, 33)
    ti = (t_emb @ dense_im).astype(np.float32)
    psi = (t_emb @ psi_kernel).astype(np.float32)
    w_t, b_t = psi[:, :COUT], psi[:, COUT:]
    E = np.einsum("ij,bj,oj->bio", conv_kernel, w_t, tc_weights).astype(np.float32)
    e = ((conv_bias * w_t) @ tc_weights.T + b_t).astype(np.float32)      # (B, 64)

    Wcat = np.concatenate([spec_w_real, spec_w_imag], axis=2)            # (33, 64, 128)
    wm = np.ascontiguousarray(Wcat.transpose(1, 0, 2).reshape(CIN, M * 128)).astype(NP_BF16)
    cstp = np.ascontiguousarray(
        CSt.reshape(NCHUNK, 128, KC).transpose(1, 0, 2)).astype(NP_BF16)  # (128,u,66)
    abt = ABt.astype(NP_BF16)
    idm = np.eye(64, dtype=np.float32)
    bnp = np.stack([np.tile(bn_scale, 2), np.tile(bn_bias, 2)], axis=1).astype(np.float32)

    x16 = x.astype(NP_BF16)
    in_maps = []
    for c in range(NCORES):
        sl = slice(BLOC * c, BLOC * (c + 1))
        xs = x16[sl]                                             # (4, L, 64) bf16
        # per chunk u: [x0 | x1 | cst | x2 | x3] as [128, u, 322]
        xr = xs.reshape(BLOC, NCHUNK, 128, CIN).transpose(0, 2, 1, 3)  # (b,128,u,64)
        xqa = np.empty((128, NCHUNK, CW), NP_BF16)
        xqa[:, :, 0:64] = xr[0]
        xqa[:, :, 64:128] = xr[1]
        xqa[:, :, 128:194] = cstp
        xqa[:, :, 194:258] = xr[2]
        xqa[:, :, 258:322] = xr[3]
        xq = np.ascontiguousarray(xqa.reshape(128, NCHUNK * CW))
        xt = np.ascontiguousarray(xs.transpose(0, 2, 1))         # (4, 64, L) bf16
        trc, tic = tr[sl], ti[sl]                                # (4, 33)
        tmod = np.concatenate([
            trc.T.reshape(-1), tic.T.reshape(-1)                 # [4k+b] each
        ]).astype(np.float32)
        tm = np.broadcast_to(tmod, (COUT, 2 * 4 * M)).copy()
        Ec = E[sl]                                               # (4, 64, 64)
        ec = e[sl]                                               # (4, 64)
        ep = np.stack([
            np.concatenate([ec[0], ec[1]]),
            np.concatenate([ec[2], ec[3]]),
        ], axis=1).astype(np.float32)                            # (128, 2)
        Ecat = np.ascontiguousarray(Ec.transpose(1, 0, 2).reshape(CIN, BLOC * COUT))
        in_maps.append({
            "xq": xq,
            "xt": xt,
            "abt": abt,
            "wm": wm,
            "ebf": Ecat.astype(NP_BF16),
            "ef": Ecat.astype(np.float32),
            "tm": tm,
            "e4": np.ascontiguousarray(ec.T).astype(np.float32),
            "ep": ep,
            "bnp": bnp,
            "idm": idm,
        })
    return in_maps


def kernel(**inputs):
    inputs = {k: np.asarray(v) for k, v in inputs.items()}
    nc = _get_nc()
    in_maps = _host_prep(**inputs)
    res = bass_utils.run_bass_kernel_spmd(
        nc, in_maps, core_ids=list(range(NCORES)),
        trace=bool(int(os.environ.get("KBENCH_TRACE", "0"))),
    )
    out = np.empty((B, L, COUT), np.float32)
    for c in range(NCORES):
        o = res.results[c]["out"].astype(np.float32)     # (4, 64, L)
        out[BLOC * c:BLOC * (c + 1)] = np.ascontiguousarray(o.transpose(0, 2, 1))
    _NC_CACHE["last_results"] = res
    return out


# revision 16
# speedup vs baseline: 1.0226x; 1.0226x over previous
"""Trainium2 Bass kernel for CTUNOBlock1D (spectral conv + time conv + batchnorm + relu).

Strategy (data-parallel over batch, 8 cores, 4 batches/core):
  - rfft uses only 33 modes -> DFT as matmuls against trig tables. x is
    shipped per l-chunk as [x0|x1|cst|x2|x3] so ONE matmul per batch-pair
    per chunk computes both the DFT (X = x^T cst) and the Gram blocks
    (x^T x, for exact BN stats via Parseval) with a 128-wide stationary.
  - mode mixing packs [Wr_k|Wi_k] into one 128-col stationary -> 33 matmuls.
  - BN stats are computed in mode space and AllReduced (64x2 f32) as early
    as possible; the collective overlaps the inverse/residual phase.
  - residual branch folded on host: E_b = K diag(w_t) Wt^T; device computes
    out^T = [Z;E]^T @ [ABt; x^T] per batch (channel-major, K=128 concat).
    PSUM drains are split across Scalar/Vector/Pool engines.
  - BN apply + ReLU is split across Scalar/Vector/Pool with per-chunk
    streaming DMA writes; output is transposed on host.
"""

import os
import numpy as np

import concourse.bass as bass
import concourse.mybir as mybir
import concourse.bacc as bacc
import concourse.tile as tile
from concourse import bass_utils

F32 = mybir.dt.float32
BF16 = mybir.dt.bfloat16
NP_BF16 = mybir.dt.np(BF16)

B, L, CIN, COUT, TEMB = 32, 8192, 64, 64, 256
M = 33            # retained rfft modes
KC = 2 * M        # 66 (real|imag concat)
NCORES = 8
BLOC = B // NCORES   # 4 batches per core
EPS = 1e-5
NCHUNK = L // 128    # 64 l-chunks of 128
CW = 322             # per-chunk xq cols: x0|x1|cst|x2|x3 = 64+64+66+64+64
USE_AR = bool(int(os.environ.get("KBENCH_AR", "1")))
USE_AG = bool(int(os.environ.get("KBENCH_AG", "0")))
USE_RD = bool(int(os.environ.get("KBENCH_RD", "0")))
WARM_CC = bool(int(os.environ.get("KBENCH_WARMCC", "0")))
NXPIECE = 8          # xq DMA split


def _build():
    nc = bacc.Bacc(None, target_bir_lowering=False)

    xq_d = nc.dram_tensor("xq", [128, NCHUNK * CW], BF16, kind="ExternalInput")
    xt_d = nc.dram_tensor("xt", [BLOC, CIN, L], BF16, kind="ExternalInput")
    abt_d = nc.dram_tensor("abt", [CIN, L], BF16, kind="ExternalInput")
    wm_d = nc.dram_tensor("wm", [CIN, M * 128], BF16, kind="ExternalInput")
    ebf_d = nc.dram_tensor("ebf", [CIN, BLOC * COUT], BF16, kind="ExternalInput")
    ef_d = nc.dram_tensor("ef", [CIN, BLOC * COUT], F32, kind="ExternalInput")
    tm_d = nc.dram_tensor("tm", [COUT, 2 * 4 * M], F32, kind="ExternalInput")
    e4_d = nc.dram_tensor("e4", [COUT, BLOC], F32, kind="ExternalInput")
    ep_d = nc.dram_tensor("ep", [128, 2], F32, kind="ExternalInput")
    bnp_d = nc.dram_tensor("bnp", [128, 2], F32, kind="ExternalInput")
    id_d = nc.dram_tensor("idm", [64, 64], F32, kind="ExternalInput")
    out_d = nc.dram_tensor("out", [BLOC, COUT, L], BF16, kind="ExternalOutput")

    rd_ref = {}
    with tile.TileContext(nc) as tc:
        with (
            tc.tile_pool(name="const", bufs=1) as cpool,
            tc.tile_pool(name="xs", bufs=1) as xpool,
            tc.tile_pool(name="xtp", bufs=1) as xtpool,
            tc.tile_pool(name="outb", bufs=1) as opool,
            tc.tile_pool(name="small", bufs=2) as spool,
            tc.tile_pool(name="psA", bufs=1, space=bass.MemorySpace.PSUM) as psA,
            tc.tile_pool(name="psS", bufs=2, space=bass.MemorySpace.PSUM) as psS,
            tc.tile_pool(name="psB", bufs=3, space=bass.MemorySpace.PSUM) as psB,
            tc.tile_pool(name="dram", bufs=1, space=bass.MemorySpace.DRAM) as dpool,
        ):
            dma = nc.sync.dma_start
            TT = nc.vector.tensor_tensor
            TS = nc.vector.tensor_scalar
            gTT = nc.gpsimd.tensor_tensor
            gTS = nc.gpsimd.tensor_scalar
            OP = mybir.AluOpType

            xq = xpool.tile([128, NCHUNK * CW], BF16, tag="xq")
            PW = NCHUNK // NXPIECE * CW
            for p in range(NXPIECE):
                dma(xq[:, PW * p:PW * (p + 1)], xq_d[:, PW * p:PW * (p + 1)])

            # small constants (own ring position; cheap)
            wm_s = cpool.tile([CIN, M * 128], BF16)
            tm_s = cpool.tile([COUT, 2 * 4 * M], F32)
            e4_s = cpool.tile([COUT, BLOC], F32)
            ep_s = cpool.tile([128, 2], F32)
            bnp_s = cpool.tile([128, 2], F32)
            id_s = cpool.tile([64, 64], F32)
            idb_s = cpool.tile([64, 64], BF16)
            ones_s = cpool.tile([64, 1], F32)
            dma(wm_s[:], wm_d[:])
            dma(tm_s[:], tm_d[:])
            dma(e4_s[:], e4_d[:])
            dma(ep_s[:], ep_d[:])
            dma(bnp_s[:], bnp_d[:])
            dma(id_s[:], id_d[:])
            nc.vector.tensor_copy(idb_s[:], id_s[:])
            nc.vector.memset(ones_s[:], 1.0)

            ebf_s = cpool.tile([CIN, BLOC * COUT], BF16)   # [i, 64b+o]
            ef_s = cpool.tile([CIN, BLOC * COUT], F32)
            zeb = cpool.tile([128, BLOC * COUT], BF16)     # [0:64]=Z^T, [64:128]=E
            dma(ebf_s[:], ebf_d[:])
            dma(ef_s[:], ef_d[:])
            dma(zeb[64:128, :], ebf_d[:])

            # early dummy collective: absorbs the cross-core launch barrier
            # and warms the CC rings while compute proceeds
            if WARM_CC and (USE_AR or USE_AG) and not USE_RD:
                wtin = dpool.tile([2, 2], F32)
                wtout = dpool.tile([2, 2], F32)
                wsrc = spool.tile([2, 2], F32, tag="wsrc")
                nc.vector.memset(wsrc[:], 0.0)
                nc.gpsimd.dma_start(wtin[:], wsrc[:])
                nc.gpsimd.collective_compute(
                    "AllReduce", mybir.AluOpType.add,
                    replica_groups=[list(range(NCORES))],
                    ins=[wtin.opt()], outs=[wtout.opt()],
                )

            # early dummy Sqrt to pre-load the ACT table set, and a dummy
            # gpsimd op to pre-load the Pool ext-isa library
            warm = spool.tile([1, 1], F32)
            nc.vector.memset(warm[:], 1.0)
            nc.scalar.activation(warm[:], warm[:], mybir.ActivationFunctionType.Sqrt)
            warm2 = spool.tile([1, 1], F32, tag="warm2")
            nc.gpsimd.tensor_tensor(warm2[:], warm[:], warm[:], mybir.AluOpType.add)

            # bulk phase-C inputs, gated behind the critical xq loads: the
            # dummy read of the last xq piece makes the sync ring wait before
            # issuing these transfers (keeps HBM bandwidth on the fwd path).
            gate = spool.tile([1, 2], BF16, tag="gate")
            dma(gate[:], xq[0:1, NCHUNK * CW - 2:NCHUNK * CW])
            xtc = []
            for b in range(BLOC):
                xt = xtpool.tile([128, L], BF16, tag=f"xtc{b}")
                xtc.append(xt)
            dma(xtc[0][0:64, :], abt_d[:])
            for b in range(BLOC):
                dma(xtc[b][64:128, :], xt_d[b])
            # replicate the ABt table into the other xtc tops off the sync ring
            nc.scalar.dma_start(xtc[1][0:64, :], xtc[0][0:64, :])
            nc.scalar.dma_start(xtc[2][0:64, :], xtc[0][0:64, :])
            nc.scalar.dma_start(xtc[3][0:64, :], xtc[1][0:64, :])

            # ---- phase A: one matmul per batch-pair per chunk ----
            # pair0: [x0|x1]^T @ [x0|x1|cst] -> [G00 G01; G10 G11 | X0; X1]
            # pair1: [x2|x3]^T @ [cst|x2|x3] -> [X2; X3 | G22 .. ; .. G33]
            pA0 = psA.tile([128, 194], F32, tag="pA0")
            pA1 = psA.tile([128, 194], F32, tag="pA1")
            for u in range(NCHUNK):
                o = CW * u
                nc.tensor.matmul(pA0[:], xq[:, o:o + 128], xq[:, o:o + 194],
                                 start=(u == 0), stop=(u == NCHUNK - 1))
                nc.tensor.matmul(pA1[:], xq[:, o + 194:o + 322], xq[:, o + 128:o + 322],
                                 start=(u == 0), stop=(u == NCHUNK - 1))

            # ---- copies out of phase-A PSUM (split across engines) ----
            Xsb = cpool.tile([CIN, BLOC * KC], BF16)    # [c, 66b+(ri,k)]
            Gsb = cpool.tile([CIN, BLOC * CIN], BF16)   # [c, 64b+c']
            nc.scalar.copy(Xsb[:, 0:66], pA0[0:64, 128:194])
            nc.vector.tensor_copy(Xsb[:, 66:132], pA0[64:128, 128:194])
            nc.scalar.copy(Xsb[:, 132:198], pA1[0:64, 0:66])
            nc.scalar.copy(Xsb[:, 198:264], pA1[64:128, 0:66])
            nc.vector.tensor_copy(Gsb[:, 0:64], pA0[0:64, 0:64])
            nc.vector.tensor_copy(Gsb[:, 64:128], pA0[64:128, 64:128])
            nc.scalar.copy(Gsb[:, 128:192], pA1[0:64, 66:130])
            nc.vector.tensor_copy(Gsb[:, 192:256], pA1[64:128, 130:194])

            # ---- phase B: mode mixing, one matmul per mode ----
            # P[0:64,8k+(j,b)] = Wr_k^T @ [Xr|Xi]; P[64:128,...] = Wi_k^T @ ...
            Pp = psS.tile([128, M * 8], F32, tag="ps_s")
            Xr4 = Xsb[:].rearrange("p (b j k) -> p j b k", b=BLOC, j=2, k=M)
            for k in range(M):
                nc.tensor.matmul(Pp[:, 8 * k:8 * (k + 1)],
                                 wm_s[:, 128 * k:128 * (k + 1)],
                                 Xr4[:, :, :, k], start=True, stop=True)

            # Gm / m1p matmuls (independent of P; share the PE queue)
            gmp = psS.tile([COUT, BLOC * KC], F32, tag="ps_s")
            m1p = psS.tile([CIN, BLOC * COUT], F32, tag="ps_s")
            for b in range(BLOC):
                nc.tensor.matmul(gmp[:, KC * b:KC * (b + 1)],
                                 ebf_s[:, 64 * b:64 * (b + 1)],
                                 Xsb[:, KC * b:KC * (b + 1)], start=True, stop=True)
                nc.tensor.matmul(m1p[:, 64 * b:64 * (b + 1)],
                                 Gsb[:, 64 * b:64 * (b + 1)],
                                 ebf_s[:, 64 * b:64 * (b + 1)], start=True, stop=True)

            Psb = spool.tile([COUT, 2 * M * 8], F32, tag="psb")
            nc.scalar.copy(Psb[:, 0:M * 8], Pp[0:64, :])
            nc.vector.tensor_copy(Psb[:, M * 8:2 * M * 8], Pp[64:128, :])

            Gm = cpool.tile([COUT, BLOC * KC], F32)     # [o, 66b+33ri+k]
            nc.scalar.copy(Gm[:], gmp[:])
            em = spool.tile([CIN, BLOC * COUT], F32, tag="em")
            TT(em[:], m1p[:], ef_s[:], OP.mult)
            qp = psS.tile([COUT, BLOC], F32, tag="ps_s")
            for b in range(BLOC):
                nc.tensor.matmul(qp[:, b:b + 1], em[:, 64 * b:64 * (b + 1)],
                                 ones_s[:], start=True, stop=True)

            Yr = spool.tile([COUT, 4 * M], F32, tag="yr")   # [(k,b)] = 4k+b
            Yi = spool.tile([COUT, 4 * M], F32, tag="yi")
            Pk1 = Psb[:, 0:M * 8].rearrange("p (k x) -> p k x", k=M, x=8)
            Pk2 = Psb[:, M * 8:2 * M * 8].rearrange("p (k x) -> p k x", k=M, x=8)
            Yrv = Yr[:].rearrange("p (k b) -> p k b", k=M, b=4)
            Yiv = Yi[:].rearrange("p (k b) -> p k b", k=M, b=4)
            TT(Yrv, Pk1[:, :, 0:4], Pk2[:, :, 4:8], OP.subtract)
            gTT(Yiv, Pk2[:, :, 0:4], Pk1[:, :, 4:8], OP.add)
            # Yr used by DVE (t1,t4), Yi by Pool (t2,t3) -- minimal cross-hops

            Zsb = cpool.tile([COUT, 2 * 4 * M], F32)  # [(ri,k,b)] = 132ri+4k+b
            t1 = spool.tile([COUT, 4 * M], F32, tag="t1")
            t2 = spool.tile([COUT, 4 * M], F32, tag="t2")
            t3 = spool.tile([COUT, 4 * M], F32, tag="t3")
            t4 = spool.tile([COUT, 4 * M], F32, tag="t4")
            ntm = 4 * M
            TT(t1[:], Yr[:], tm_s[:, 0:ntm], OP.mult)
            TT(t4[:], Yr[:], tm_s[:, ntm:2 * ntm], OP.mult)
            gTT(t2[:], Yi[:], tm_s[:, ntm:2 * ntm], OP.mult)
            gTT(t3[:], Yi[:], tm_s[:, 0:ntm], OP.mult)
            TT(Zsb[:, 0:ntm], t1[:], t2[:], OP.subtract)
            TT(Zsb[:, ntm:2 * ntm], t3[:], t4[:], OP.add)

            # ---- stats in mode space (batched over the 4 batches) ----
            q4 = spool.tile([COUT, BLOC], F32, tag="q4")
            A12 = spool.tile([COUT, BLOC], F32, tag="a12")
            nc.vector.tensor_copy(q4[:], qp[:])
            # A12 = sum_k>=1 Zr*(Zr+2Gr) + Zi*(Zi+2Gi)  (= A1 + 2*A2)
            Zall = Zsb[:].rearrange("p (ri k b) -> p b ri k", ri=2, k=M, b=4)[:, :, :, 1:M]
            Gall = Gm[:].rearrange("p (b ri k) -> p b ri k", b=BLOC, ri=2, k=M)[:, :, :, 1:M]
            w256a = spool.tile([COUT, BLOC * 64], F32, tag="w256a")
            w256b = spool.tile([COUT, BLOC * 64], F32, tag="w256b")
            wa = w256a[:].rearrange("p (b ri k) -> p b ri k", b=4, ri=2, k=M - 1)
            wb = w256b[:].rearrange("p (b ri k) -> p b ri k", b=4, ri=2, k=M - 1)
            gTS(wa, Gall, 2.0, 0.0, OP.mult, OP.add)
            TT(wb, Zall, wa, OP.add)
            TT(wa, Zall, wb, OP.mult)
            nc.vector.tensor_reduce(
                A12[:], w256a[:].rearrange("p (b k) -> p b k", b=BLOC, k=64),
                mybir.AxisListType.X, OP.add)

            # vectorized S1/S2 assembly over the 4 batches
            Zr04 = Zsb[:, 0:4]                                  # Zr[k=0] per b
            u4 = Gm[:].rearrange("p (b x) -> p b x", b=BLOC, x=KC)[:, :, 0]
            v4 = spool.tile([COUT, BLOC], F32, tag="v4")
            s2c = spool.tile([COUT, BLOC], F32, tag="s2c")
            w1 = spool.tile([COUT, BLOC], F32, tag="w1")
            w2 = spool.tile([COUT, BLOC], F32, tag="w2")
            TT(v4[:], Zr04, u4, OP.add)
            TT(v4[:], v4[:], e4_s[:], OP.add)                   # v = Zr0+u+e
            gTT(w2[:], Zr04, u4, OP.mult)                       # Zr0*u
            TT(s2c[:], Zr04, Zr04, OP.mult)                     # Zr0^2
            TS(w1[:], A12[:], 2.0, 0.0, OP.mult, OP.add)
            TT(s2c[:], s2c[:], w1[:], OP.add)
            TS(w1[:], q4[:], 1.0 / L, 0.0, OP.mult, OP.add)
            TT(s2c[:], s2c[:], w1[:], OP.add)
            TT(w1[:], e4_s[:], v4[:], OP.mult)
            TT(w1[:], w1[:], w2[:], OP.add)                     # e*v + Zr0*u
            TS(w1[:], w1[:], 2.0, 0.0, OP.mult, OP.add)
            TT(s2c[:], s2c[:], w1[:], OP.add)
            TT(w1[:], e4_s[:], e4_s[:], OP.mult)
            TT(s2c[:], s2c[:], w1[:], OP.subtract)

            stat_in = spool.tile([COUT, 2], F32, tag="stin")
            nc.vector.tensor_reduce(stat_in[:, 0:1], v4[:], mybir.AxisListType.X, OP.add)
            nc.vector.tensor_reduce(stat_in[:, 1:2], s2c[:], mybir.AxisListType.X, OP.add)

            # ---- cross-core reduction of (64,2) stats ----
            st128 = spool.tile([128, 2], F32, tag="st128")
            if USE_RD:
                # P2P stats exchange over remote SBUF-to-SBUF DMA: each core
                # broadcasts its 512B stats to the 7 peers (slot j written by
                # peer me^j), then reduces locally once 7x2 sem bumps land.
                rsem = nc.alloc_semaphore(name="rd_rsem")
                lsem = nc.alloc_semaphore(name="rd_lsem")
                st_loc = cpool.tile([128, 1], F32)   # v on 0:64, s2 on 64:128
                nc.scalar.copy(st_loc[0:64, :], stat_in[:, 0:1])
                nc.scalar.copy(st_loc[64:128, :], stat_in[:, 1:2])
                rbuf = cpool.tile([128, 8], F32)
                nc.vector.tensor_copy(rbuf[:, 0:1], st_loc[:])   # self slot 0
                for dt in range(1, NCORES):
                    rdests = [None] * 8
                    rdests[dt] = (0, dt)
                    nc.gpsimd.remote_dma_broadcast(
                        rbuf[:, dt:dt + 1], st_loc[:], rsem, lsem, rdests=rdests)
                nc.gpsimd.trigger_dma(count=None)
                red = spool.tile([128, 1], F32, tag="red")
                ri = nc.vector.tensor_reduce(red[:], rbuf[:],
                                             mybir.AxisListType.X, OP.add)
                # the remote-sem wait is attached AFTER Tile scheduling (the
                # scheduling sim cannot model peer increments and deadlocks)
                rd_ref["ri"] = ri
                rd_ref["rsem"] = rsem
                nc.scalar.copy(st128[0:64, 0:1], red[0:64, :])
                nc.scalar.copy(st128[64:128, 0:1], red[0:64, :])
                nc.scalar.copy(st128[0:64, 1:2], red[64:128, :])
                nc.scalar.copy(st128[64:128, 1:2], red[64:128, :])
            elif USE_AG:
                din = dpool.tile([COUT, 2], F32)
                dout = dpool.tile([NCORES, COUT * 2], F32)
                nc.gpsimd.dma_start(din[:], stat_in[:])
                nc.gpsimd.collective_compute(
                    "AllGather", OP.bypass,
                    replica_groups=[list(range(NCORES))],
                    ins=[din.opt()], outs=[dout.opt()],
                )
                allst = spool.tile([COUT, NCORES * 2], F32, tag="allst")
                nc.scalar.dma_start(
                    allst[:].rearrange("p (r s) -> p r s", r=NCORES, s=2),
                    dout[:].rearrange("r (p s) -> p r s", p=COUT, s=2))
                st64 = spool.tile([COUT, 2], F32, tag="st64")
                av = allst[:].rearrange("p (r s) -> p s r", r=NCORES, s=2)
                nc.vector.tensor_reduce(st64[:], av, mybir.AxisListType.X, OP.add)
                nc.scalar.dma_start(st128[0:64, :], st64[:])
                nc.scalar.dma_start(st128[64:128, :], st64[:])
            else:
                din = dpool.tile([COUT, 2], F32)
                dout = dpool.tile([COUT, 2], F32)
                nc.gpsimd.dma_start(din[:], stat_in[:])
                if USE_AR:
                    nc.gpsimd.collective_compute(
                        "AllReduce", OP.add,
                        replica_groups=[list(range(NCORES))],
                        ins=[din.opt()], outs=[dout.opt()],
                    )
                else:
                    nc.gpsimd.dma_start(dout[:], din[:])
                nc.scalar.dma_start(st128[0:64, :], dout[:])
                nc.scalar.dma_start(st128[64:128, :], dout[:])

            # ---- Z transpose (per batch, DC row folded into bias later) ----
            # zeb[b]: rows 0:64 = Z^T (modes k=1..32, re|im), rows 64:128 = E_b
            Zflat = spool.tile([COUT, 4 * 64], BF16, tag="zflat")  # [b][ri,k>=1]
            nc.scalar.copy(
                Zflat[:].rearrange("p (b ri k) -> p b ri k", b=4, ri=2, k=M - 1),
                Zsb[:].rearrange("p (ri k b) -> p b ri k", ri=2, k=M, b=4)[:, :, :, 1:M])
            for b in range(BLOC):
                tp = psA.tile([CIN, COUT], BF16, tag="ps_tp")
                nc.tensor.transpose(tp[:], Zflat[:, 64 * b:64 * (b + 1)], idb_s[:])
                nc.scalar.copy(zeb[0:64, 64 * b:64 * (b + 1)], tp[:])

            # Zr0 (DC) pair-stacking via partition-shifted engine copies
            zr0p = spool.tile([128, 2], F32, tag="zr0p")
            zr0v = zr0p[:].rearrange("p (j a) -> p j a", j=2, a=1)
            nc.scalar.copy(zr0v[0:64, :, 0],
                           Zsb[:, 0:4].rearrange("p (k b) -> p b k", k=1, b=4)[:, 0:4:2, 0])
            nc.scalar.copy(zr0v[64:128, :, 0],
                           Zsb[:, 0:4].rearrange("p (k b) -> p b k", k=1, b=4)[:, 1:4:2, 0])
            epz = spool.tile([128, 2], F32, tag="epz")
            nc.scalar.add = nc.scalar.add  # noqa
            TT(epz[:], ep_s[:], zr0p[:], OP.add)

            # ---- phase C: single K=128 matmul per tile: [Z;E]^T @ [ABt;xT] ----
            OUT = []
            for j in range(2):
                outj = opool.tile([128, L], BF16, tag=f"out{j}")
                OUT.append(outj)

            NSTEP = 512
            drains = [nc.vector.tensor_copy, nc.scalar.copy]
            di = 0
            for j in range(2):
                b0, b1 = 2 * j, 2 * j + 1
                for n in range(L // NSTEP):
                    ps = psB.tile([128, NSTEP], F32, tag="invres")
                    sl = slice(NSTEP * n, NSTEP * (n + 1))
                    nc.tensor.matmul(ps[0:64, :], zeb[:, 64 * b0:64 * b0 + 64],
                                     xtc[b0][:, sl], start=True, stop=True)
                    nc.tensor.matmul(ps[64:128, :], zeb[:, 64 * b1:64 * b1 + 64],
                                     xtc[b1][:, sl], start=True, stop=True,
                                     tile_position=(0, 64))
                    drains[di % 2](OUT[j][:, sl], ps[:])
                    di += 1

            # ---- BN scale/shift from all-reduced stats ----
            mean = spool.tile([128, 1], F32, tag="mean")
            ex2 = spool.tile([128, 1], F32, tag="ex2")
            var = spool.tile([128, 1], F32, tag="var")
            sv = spool.tile([128, 1], F32, tag="sv")
            sh = spool.tile([128, 1], F32, tag="sh")
            wk = spool.tile([128, 1], F32, tag="wk")
            TS(mean[:], st128[:, 0:1], 1.0 / B, 0.0, OP.mult, OP.add)
            TS(ex2[:], st128[:, 1:2], 1.0 / B, 0.0, OP.mult, OP.add)
            TT(wk[:], mean[:], mean[:], OP.mult)
            TT(var[:], ex2[:], wk[:], OP.subtract)
            TS(var[:], var[:], 1.0, EPS, OP.mult, OP.add)
            nc.scalar.activation(wk[:], var[:], mybir.ActivationFunctionType.Sqrt)
            nc.vector.reciprocal(sv[:], wk[:])
            TT(sv[:], sv[:], bnp_s[:, 0:1], OP.mult)            # s = bn_scale/std
            TT(wk[:], mean[:], sv[:], OP.mult)
            TT(sh[:], bnp_s[:, 1:2], wk[:], OP.subtract)        # shift = bias - mean*s

            bjs = []
            for j in range(2):
                bj = spool.tile([128, 1], F32, tag=f"bj{j}")
                TT(bj[:], epz[:, j:j + 1], sv[:], OP.mult)      # s*(e_b + Zr0)
                TT(bj[:], bj[:], sh[:], OP.add)                 # + shift
                bjs.append(bj)

            # ---- apply BN+ReLU split over ACT(1-pass) / DVE(2-pass, 2x bf16),
            # streaming each applied chunk out on alternating DMA rings ----
            NQ = 1024
            plan = {}
            acts = {(0, 0), (0, 1), (1, 0), (2, 1), (4, 0), (5, 1), (6, 0)}
            wrings = [nc.sync.dma_start, nc.scalar.dma_start]
            for n2 in range(8):
                for j in range(2):
                    q = slice(n2 * NQ, (n2 + 1) * NQ)
                    if (n2, j) in acts:
                        nc.scalar.activation(OUT[j][:, q], OUT[j][:, q],
                                             mybir.ActivationFunctionType.Relu,
                                             bias=bjs[j][:], scale=sv[:])
                    else:
                        TS(OUT[j][:, q], OUT[j][:, q], sv[:], bjs[j][:],
                           OP.mult, OP.add)
                        TS(OUT[j][:, q], OUT[j][:, q], 0.0, 0.0, OP.max, OP.add)
                    od = out_d[2 * j:2 * j + 2].rearrange("a b l -> (a b) l")
                    wrings[(2 * n2 + j) % 2](od[:, q], OUT[j][:, q])

    if rd_ref:
        rd_ref["ri"].wait_op(rd_ref["rsem"], 2 * (NCORES - 1), "sem-ge", check=False)
    nc.compile()
    return nc


_NC_CACHE = {}


def _get_nc():
    if "nc" not in _NC_CACHE:
        _NC_CACHE["nc"] = _build()
    return _NC_CACHE["nc"]


def _host_prep(x, t_emb, spec_w_real, spec_w_imag, dense_re, dense_im,
               conv_kernel, conv_bias, tc_weights, psi_kernel, bn_scale, bn_bias):
    """Build per-core input maps (small tensors precomputed on host)."""
    k = np.arange(M)
    l = np.arange(L)
    ang = 2.0 * np.pi * np.outer(l, k) / L
    CSt = np.concatenate([np.cos(ang) / L, -np.sin(ang) / L], axis=1)   # (L, 66)
    angk = ang[:, 1:]                                # drop DC mode
    ABt = np.concatenate([(2.0 * np.cos(angk)).T,
                          (-2.0 * np.sin(angk)).T], axis=0).astype(np.float32)

    tr = (t_emb @ dense_re).astype(np.float32)      # (B, 33)
    ti = (t_emb @ dense_im).astype(np.float32)
    psi = (t_emb @ psi_kernel).astype(np.float32)
    w_t, b_t = psi[:, :COUT], psi[:, COUT:]
    E = np.einsum("ij,bj,oj->bio", conv_kernel, w_t, tc_weights).astype(np.float32)
    e = ((conv_bias * w_t) @ tc_weights.T + b_t).astype(np.float32)      # (B, 64)

    Wcat = np.concatenate([spec_w_real, spec_w_imag], axis=2)            # (33, 64, 128)
    wm = np.ascontiguousarray(Wcat.transpose(1, 0, 2).reshape(CIN, M * 128)).astype(NP_BF16)
    cstp = np.ascontiguousarray(
        CSt.reshape(NCHUNK, 128, KC).transpose(1, 0, 2)).astype(NP_BF16)  # (128,u,66)
    abt = ABt.astype(NP_BF16)
    idm = np.eye(64, dtype=np.float32)
    bnp = np.stack([np.tile(bn_scale, 2), np.tile(bn_bias, 2)], axis=1).astype(np.float32)

    x16 = x.astype(NP_BF16)
    in_maps = []
    for c in range(NCORES):
        sl = slice(BLOC * c, BLOC * (c + 1))
        xs = x16[sl]                                             # (4, L, 64) bf16
        # per chunk u: [x0 | x1 | cst | x2 | x3] as [128, u, 322]
        xr = xs.reshape(BLOC, NCHUNK, 128, CIN).transpose(0, 2, 1, 3)  # (b,128,u,64)
        xqa = np.empty((128, NCHUNK, CW), NP_BF16)
        xqa[:, :, 0:64] = xr[0]
        xqa[:, :, 64:128] = xr[1]
        xqa[:, :, 128:194] = cstp
        xqa[:, :, 194:258] = xr[2]
        xqa[:, :, 258:322] = xr[3]
        xq = np.ascontiguousarray(xqa.reshape(128, NCHUNK * CW))
        xt = np.ascontiguousarray(xs.transpose(0, 2, 1))         # (4, 64, L) bf16
        trc, tic = tr[sl], ti[sl]                                # (4, 33)
        tmod = np.concatenate([
            trc.T.reshape(-1), tic.T.reshape(-1)                 # [4k+b] each
        ]).astype(np.float32)
        tm = np.broadcast_to(tmod, (COUT, 2 * 4 * M)).copy()
        Ec = E[sl]                                               # (4, 64, 64)
        ec = e[sl]                                               # (4, 64)
        ep = np.stack([
            np.concatenate([ec[0], ec[1]]),
            np.concatenate([ec[2], ec[3]]),
        ], axis=1).astype(np.float32)                            # (128, 2)
        Ecat = np.ascontiguousarray(Ec.transpose(1, 0, 2).reshape(CIN, BLOC * COUT))
        in_maps.append({
            "xq": xq,
            "xt": xt,
            "abt": abt,
            "wm": wm,
            "ebf": Ecat.astype(NP_BF16),
            "ef": Ecat.astype(np.float32),
            "tm": tm,
            "e4": np.ascontiguousarray(ec.T).astype(np.float32),
            "ep": ep,
            "bnp": bnp,
            "idm": idm,
        })
    return in_maps


def kernel(**inputs):
    inputs = {k: np.asarray(v) for k, v in inputs.items()}
    nc = _get_nc()
    in_maps = _host_prep(**inputs)
    res = bass_utils.run_bass_kernel_spmd(
        nc, in_maps, core_ids=list(range(NCORES)),
        trace=bool(int(os.environ.get("KBENCH_TRACE", "0"))),
    )
    out = np.empty((B, L, COUT), np.float32)
    for c in range(NCORES):
        o = res.results[c]["out"].astype(np.float32)     # (4, 64, L)
        out[BLOC * c:BLOC * (c + 1)] = np.ascontiguousarray(o.transpose(0, 2, 1))
    _NC_CACHE["last_results"] = res
    return out


# revision 17
# speedup vs baseline: 1.0475x; 1.0244x over previous
"""Trainium2 Bass kernel for CTUNOBlock1D (spectral conv + time conv + batchnorm + relu).

Strategy (data-parallel over batch, 8 cores, 4 batches/core):
  - rfft uses only 33 modes -> DFT as matmuls against trig tables. x is
    shipped per l-chunk as [x0|x1|cst|x2|x3] so ONE matmul per batch-pair
    per chunk computes both the DFT (X = x^T cst) and the Gram blocks
    (x^T x, for exact BN stats via Parseval) with a 128-wide stationary.
  - mode mixing packs [Wr_k|Wi_k] into one 128-col stationary -> 33 matmuls.
  - BN stats are computed in mode space and AllReduced (64x2 f32) as early
    as possible; the collective overlaps the inverse/residual phase.
  - residual branch folded on host: E_b = K diag(w_t) Wt^T; device computes
    out^T = [Z;E]^T @ [ABt; x^T] per batch (channel-major, K=128 concat).
    PSUM drains are split across Scalar/Vector/Pool engines.
  - BN apply + ReLU is split across Scalar/Vector/Pool with per-chunk
    streaming DMA writes; output is transposed on host.
"""

import os
import numpy as np

import concourse.bass as bass
import concourse.mybir as mybir
import concourse.bacc as bacc
import concourse.tile as tile
from concourse import bass_utils

F32 = mybir.dt.float32
BF16 = mybir.dt.bfloat16
NP_BF16 = mybir.dt.np(BF16)

B, L, CIN, COUT, TEMB = 32, 8192, 64, 64, 256
M = 33            # retained rfft modes
KC = 2 * M        # 66 (real|imag concat)
NCORES = 8
BLOC = B // NCORES   # 4 batches per core
EPS = 1e-5
NCHUNK = L // 128    # 64 l-chunks of 128
CW = 322             # per-chunk xq cols: x0|x1|cst|x2|x3 = 64+64+66+64+64
USE_AR = bool(int(os.environ.get("KBENCH_AR", "1")))
USE_AG = bool(int(os.environ.get("KBENCH_AG", "0")))
USE_RD = bool(int(os.environ.get("KBENCH_RD", "0")))
WARM_CC = bool(int(os.environ.get("KBENCH_WARMCC", "0")))
NXPIECE = 8          # xq DMA split


def _build():
    nc = bacc.Bacc(None, target_bir_lowering=False)

    xq_d = nc.dram_tensor("xq", [128, NCHUNK * CW], BF16, kind="ExternalInput")
    xt_d = nc.dram_tensor("xt", [BLOC, CIN, L], BF16, kind="ExternalInput")
    abt_d = nc.dram_tensor("abt", [CIN, L], BF16, kind="ExternalInput")
    wm_d = nc.dram_tensor("wm", [CIN, M * 128], BF16, kind="ExternalInput")
    ebf_d = nc.dram_tensor("ebf", [CIN, BLOC * COUT], BF16, kind="ExternalInput")
    ef_d = nc.dram_tensor("ef", [CIN, BLOC * COUT], F32, kind="ExternalInput")
    tm_d = nc.dram_tensor("tm", [COUT, 2 * 4 * M], F32, kind="ExternalInput")
    e4_d = nc.dram_tensor("e4", [COUT, BLOC], F32, kind="ExternalInput")
    ep_d = nc.dram_tensor("ep", [128, 2], F32, kind="ExternalInput")
    bnp_d = nc.dram_tensor("bnp", [128, 2], F32, kind="ExternalInput")
    id_d = nc.dram_tensor("idm", [64, 64], F32, kind="ExternalInput")
    out_d = nc.dram_tensor("out", [BLOC, COUT, L], BF16, kind="ExternalOutput")

    rd_ref = {}
    with tile.TileContext(nc) as tc:
        with (
            tc.tile_pool(name="const", bufs=1) as cpool,
            tc.tile_pool(name="xs", bufs=1) as xpool,
            tc.tile_pool(name="xtp", bufs=1) as xtpool,
            tc.tile_pool(name="outb", bufs=1) as opool,
            tc.tile_pool(name="small", bufs=2) as spool,
            tc.tile_pool(name="psA", bufs=1, space=bass.MemorySpace.PSUM) as psA,
            tc.tile_pool(name="psS", bufs=2, space=bass.MemorySpace.PSUM) as psS,
            tc.tile_pool(name="psB", bufs=3, space=bass.MemorySpace.PSUM) as psB,
            tc.tile_pool(name="dram", bufs=1, space=bass.MemorySpace.DRAM) as dpool,
        ):
            dma = nc.sync.dma_start
            TT = nc.vector.tensor_tensor
            TS = nc.vector.tensor_scalar
            gTT = nc.gpsimd.tensor_tensor
            gTS = nc.gpsimd.tensor_scalar
            OP = mybir.AluOpType

            xq = xpool.tile([128, NCHUNK * CW], BF16, tag="xq")
            PW = NCHUNK // NXPIECE * CW
            rings = [nc.sync.dma_start, nc.scalar.dma_start]
            for p in range(NXPIECE):
                rings[p % 2](xq[:, PW * p:PW * (p + 1)], xq_d[:, PW * p:PW * (p + 1)])

            # small constants (own ring position; cheap)
            wm_s = cpool.tile([CIN, M * 128], BF16)
            tm_s = cpool.tile([COUT, 2 * 4 * M], F32)
            e4_s = cpool.tile([COUT, BLOC], F32)
            ep_s = cpool.tile([128, 2], F32)
            bnp_s = cpool.tile([128, 2], F32)
            id_s = cpool.tile([64, 64], F32)
            idb_s = cpool.tile([64, 64], BF16)
            ones_s = cpool.tile([64, 1], F32)
            dma(wm_s[:], wm_d[:])
            dma(tm_s[:], tm_d[:])
            dma(e4_s[:], e4_d[:])
            dma(ep_s[:], ep_d[:])
            dma(bnp_s[:], bnp_d[:])
            dma(id_s[:], id_d[:])
            nc.vector.tensor_copy(idb_s[:], id_s[:])
            nc.vector.memset(ones_s[:], 1.0)

            ebf_s = cpool.tile([CIN, BLOC * COUT], BF16)   # [i, 64b+o]
            ef_s = cpool.tile([CIN, BLOC * COUT], F32)
            zeb = cpool.tile([128, BLOC * COUT], BF16)     # [0:64]=Z^T, [64:128]=E
            dma(ebf_s[:], ebf_d[:])
            dma(ef_s[:], ef_d[:])
            dma(zeb[64:128, :], ebf_d[:])

            # early dummy collective: absorbs the cross-core launch barrier
            # and warms the CC rings while compute proceeds
            if WARM_CC and (USE_AR or USE_AG) and not USE_RD:
                wtin = dpool.tile([2, 2], F32)
                wtout = dpool.tile([2, 2], F32)
                wsrc = spool.tile([2, 2], F32, tag="wsrc")
                nc.vector.memset(wsrc[:], 0.0)
                nc.gpsimd.dma_start(wtin[:], wsrc[:])
                nc.gpsimd.collective_compute(
                    "AllReduce", mybir.AluOpType.add,
                    replica_groups=[list(range(NCORES))],
                    ins=[wtin.opt()], outs=[wtout.opt()],
                )

            # early dummy Sqrt to pre-load the ACT table set, and a dummy
            # gpsimd op to pre-load the Pool ext-isa library
            warm = spool.tile([1, 1], F32)
            nc.vector.memset(warm[:], 1.0)
            nc.scalar.activation(warm[:], warm[:], mybir.ActivationFunctionType.Sqrt)
            warm2 = spool.tile([1, 1], F32, tag="warm2")
            nc.gpsimd.tensor_tensor(warm2[:], warm[:], warm[:], mybir.AluOpType.add)

            # bulk phase-C inputs, gated behind the critical xq loads: the
            # dummy read of the last xq piece makes the sync ring wait before
            # issuing these transfers (keeps HBM bandwidth on the fwd path).
            gate = spool.tile([1, 2], BF16, tag="gate")
            dma(gate[:], xq[0:1, NCHUNK * CW - 2:NCHUNK * CW])
            gate2 = spool.tile([1, 2], BF16, tag="gate2")
            nc.scalar.dma_start(gate2[:], xq[0:1, PW * 7 - 2:PW * 7])
            xtc = []
            for b in range(BLOC):
                xt = xtpool.tile([128, L], BF16, tag=f"xtc{b}")
                xtc.append(xt)
            nc.scalar.dma_start(xtc[0][0:64, :], abt_d[:])
            dma(xtc[0][64:128, :], xt_d[0])
            nc.scalar.dma_start(xtc[1][64:128, :], xt_d[1])
            dma(xtc[2][64:128, :], xt_d[2])
            nc.scalar.dma_start(xtc[3][64:128, :], xt_d[3])
            # replicate the ABt table into the other xtc tops (sync ring,
            # which is idle once xt b0/b2 have landed)
            dma(xtc[1][0:64, :], xtc[0][0:64, :])
            dma(xtc[2][0:64, :], xtc[0][0:64, :])
            dma(xtc[3][0:64, :], xtc[1][0:64, :])

            # ---- phase A: one matmul per batch-pair per chunk ----
            # pair0: [x0|x1]^T @ [x0|x1|cst] -> [G00 G01; G10 G11 | X0; X1]
            # pair1: [x2|x3]^T @ [cst|x2|x3] -> [X2; X3 | G22 .. ; .. G33]
            pA0 = psA.tile([128, 194], F32, tag="pA0")
            pA1 = psA.tile([128, 194], F32, tag="pA1")
            for u in range(NCHUNK):
                o = CW * u
                nc.tensor.matmul(pA0[:], xq[:, o:o + 128], xq[:, o:o + 194],
                                 start=(u == 0), stop=(u == NCHUNK - 1))
                nc.tensor.matmul(pA1[:], xq[:, o + 194:o + 322], xq[:, o + 128:o + 322],
                                 start=(u == 0), stop=(u == NCHUNK - 1))

            # ---- copies out of phase-A PSUM (split across engines) ----
            Xsb = cpool.tile([CIN, BLOC * KC], BF16)    # [c, 66b+(ri,k)]
            Gsb = cpool.tile([CIN, BLOC * CIN], BF16)   # [c, 64b+c']
            nc.scalar.copy(Xsb[:, 0:66], pA0[0:64, 128:194])
            nc.vector.tensor_copy(Xsb[:, 66:132], pA0[64:128, 128:194])
            nc.scalar.copy(Xsb[:, 132:198], pA1[0:64, 0:66])
            nc.scalar.copy(Xsb[:, 198:264], pA1[64:128, 0:66])
            nc.vector.tensor_copy(Gsb[:, 0:64], pA0[0:64, 0:64])
            nc.vector.tensor_copy(Gsb[:, 64:128], pA0[64:128, 64:128])
            nc.scalar.copy(Gsb[:, 128:192], pA1[0:64, 66:130])
            nc.vector.tensor_copy(Gsb[:, 192:256], pA1[64:128, 130:194])

            # ---- phase B: mode mixing, one matmul per mode ----
            # P[0:64,8k+(j,b)] = Wr_k^T @ [Xr|Xi]; P[64:128,...] = Wi_k^T @ ...
            Pp = psS.tile([128, M * 8], F32, tag="ps_s")
            Xr4 = Xsb[:].rearrange("p (b j k) -> p j b k", b=BLOC, j=2, k=M)
            for k in range(M):
                nc.tensor.matmul(Pp[:, 8 * k:8 * (k + 1)],
                                 wm_s[:, 128 * k:128 * (k + 1)],
                                 Xr4[:, :, :, k], start=True, stop=True)

            # Gm / m1p matmuls (independent of P; share the PE queue)
            gmp = psS.tile([COUT, BLOC * KC], F32, tag="ps_s")
            m1p = psS.tile([CIN, BLOC * COUT], F32, tag="ps_s")
            for b in range(BLOC):
                nc.tensor.matmul(gmp[:, KC * b:KC * (b + 1)],
                                 ebf_s[:, 64 * b:64 * (b + 1)],
                                 Xsb[:, KC * b:KC * (b + 1)], start=True, stop=True)
                nc.tensor.matmul(m1p[:, 64 * b:64 * (b + 1)],
                                 Gsb[:, 64 * b:64 * (b + 1)],
                                 ebf_s[:, 64 * b:64 * (b + 1)], start=True, stop=True)

            Psb = spool.tile([COUT, 2 * M * 8], F32, tag="psb")
            nc.scalar.copy(Psb[:, 0:M * 8], Pp[0:64, :])
            nc.vector.tensor_copy(Psb[:, M * 8:2 * M * 8], Pp[64:128, :])

            Gm = cpool.tile([COUT, BLOC * KC], F32)     # [o, 66b+33ri+k]
            nc.scalar.copy(Gm[:], gmp[:])
            em = spool.tile([CIN, BLOC * COUT], F32, tag="em")
            TT(em[:], m1p[:], ef_s[:], OP.mult)
            qp = psS.tile([COUT, BLOC], F32, tag="ps_s")
            for b in range(BLOC):
                nc.tensor.matmul(qp[:, b:b + 1], em[:, 64 * b:64 * (b + 1)],
                                 ones_s[:], start=True, stop=True)

            Yr = spool.tile([COUT, 4 * M], F32, tag="yr")   # [(k,b)] = 4k+b
            Yi = spool.tile([COUT, 4 * M], F32, tag="yi")
            Pk1 = Psb[:, 0:M * 8].rearrange("p (k x) -> p k x", k=M, x=8)
            Pk2 = Psb[:, M * 8:2 * M * 8].rearrange("p (k x) -> p k x", k=M, x=8)
            Yrv = Yr[:].rearrange("p (k b) -> p k b", k=M, b=4)
            Yiv = Yi[:].rearrange("p (k b) -> p k b", k=M, b=4)
            TT(Yrv, Pk1[:, :, 0:4], Pk2[:, :, 4:8], OP.subtract)
            gTT(Yiv, Pk2[:, :, 0:4], Pk1[:, :, 4:8], OP.add)
            # Yr used by DVE (t1,t4), Yi by Pool (t2,t3) -- minimal cross-hops

            Zsb = cpool.tile([COUT, 2 * 4 * M], F32)  # [(ri,k,b)] = 132ri+4k+b
            t1 = spool.tile([COUT, 4 * M], F32, tag="t1")
            t2 = spool.tile([COUT, 4 * M], F32, tag="t2")
            t3 = spool.tile([COUT, 4 * M], F32, tag="t3")
            t4 = spool.tile([COUT, 4 * M], F32, tag="t4")
            ntm = 4 * M
            TT(t1[:], Yr[:], tm_s[:, 0:ntm], OP.mult)
            TT(t4[:], Yr[:], tm_s[:, ntm:2 * ntm], OP.mult)
            gTT(t2[:], Yi[:], tm_s[:, ntm:2 * ntm], OP.mult)
            gTT(t3[:], Yi[:], tm_s[:, 0:ntm], OP.mult)
            TT(Zsb[:, 0:ntm], t1[:], t2[:], OP.subtract)
            TT(Zsb[:, ntm:2 * ntm], t3[:], t4[:], OP.add)

            # ---- stats in mode space (batched over the 4 batches) ----
            q4 = spool.tile([COUT, BLOC], F32, tag="q4")
            A12 = spool.tile([COUT, BLOC], F32, tag="a12")
            nc.vector.tensor_copy(q4[:], qp[:])
            # A12 = sum_k>=1 Zr*(Zr+2Gr) + Zi*(Zi+2Gi)  (= A1 + 2*A2)
            Zall = Zsb[:].rearrange("p (ri k b) -> p b ri k", ri=2, k=M, b=4)[:, :, :, 1:M]
            Gall = Gm[:].rearrange("p (b ri k) -> p b ri k", b=BLOC, ri=2, k=M)[:, :, :, 1:M]
            w256a = spool.tile([COUT, BLOC * 64], F32, tag="w256a")
            w256b = spool.tile([COUT, BLOC * 64], F32, tag="w256b")
            wa = w256a[:].rearrange("p (b ri k) -> p b ri k", b=4, ri=2, k=M - 1)
            wb = w256b[:].rearrange("p (b ri k) -> p b ri k", b=4, ri=2, k=M - 1)
            gTS(wa, Gall, 2.0, 0.0, OP.mult, OP.add)
            TT(wb, Zall, wa, OP.add)
            TT(wa, Zall, wb, OP.mult)
            nc.vector.tensor_reduce(
                A12[:], w256a[:].rearrange("p (b k) -> p b k", b=BLOC, k=64),
                mybir.AxisListType.X, OP.add)

            # vectorized S1/S2 assembly over the 4 batches
            Zr04 = Zsb[:, 0:4]                                  # Zr[k=0] per b
            u4 = Gm[:].rearrange("p (b x) -> p b x", b=BLOC, x=KC)[:, :, 0]
            v4 = spool.tile([COUT, BLOC], F32, tag="v4")
            s2c = spool.tile([COUT, BLOC], F32, tag="s2c")
            w1 = spool.tile([COUT, BLOC], F32, tag="w1")
            w2 = spool.tile([COUT, BLOC], F32, tag="w2")
            TT(v4[:], Zr04, u4, OP.add)
            TT(v4[:], v4[:], e4_s[:], OP.add)                   # v = Zr0+u+e
            gTT(w2[:], Zr04, u4, OP.mult)                       # Zr0*u
            TT(s2c[:], Zr04, Zr04, OP.mult)                     # Zr0^2
            TS(w1[:], A12[:], 2.0, 0.0, OP.mult, OP.add)
            TT(s2c[:], s2c[:], w1[:], OP.add)
            TS(w1[:], q4[:], 1.0 / L, 0.0, OP.mult, OP.add)
            TT(s2c[:], s2c[:], w1[:], OP.add)
            TT(w1[:], e4_s[:], v4[:], OP.mult)
            TT(w1[:], w1[:], w2[:], OP.add)                     # e*v + Zr0*u
            TS(w1[:], w1[:], 2.0, 0.0, OP.mult, OP.add)
            TT(s2c[:], s2c[:], w1[:], OP.add)
            TT(w1[:], e4_s[:], e4_s[:], OP.mult)
            TT(s2c[:], s2c[:], w1[:], OP.subtract)

            stat_in = spool.tile([COUT, 2], F32, tag="stin")
            nc.vector.tensor_reduce(stat_in[:, 0:1], v4[:], mybir.AxisListType.X, OP.add)
            nc.vector.tensor_reduce(stat_in[:, 1:2], s2c[:], mybir.AxisListType.X, OP.add)

            # ---- cross-core reduction of (64,2) stats ----
            st128 = spool.tile([128, 2], F32, tag="st128")
            if USE_RD:
                # P2P stats exchange over remote SBUF-to-SBUF DMA: each core
                # broadcasts its 512B stats to the 7 peers (slot j written by
                # peer me^j), then reduces locally once 7x2 sem bumps land.
                rsem = nc.alloc_semaphore(name="rd_rsem")
                lsem = nc.alloc_semaphore(name="rd_lsem")
                st_loc = cpool.tile([128, 1], F32)   # v on 0:64, s2 on 64:128
                nc.scalar.copy(st_loc[0:64, :], stat_in[:, 0:1])
                nc.scalar.copy(st_loc[64:128, :], stat_in[:, 1:2])
                rbuf = cpool.tile([128, 8], F32)
                nc.vector.tensor_copy(rbuf[:, 0:1], st_loc[:])   # self slot 0
                for dt in range(1, NCORES):
                    rdests = [None] * 8
                    rdests[dt] = (0, dt)
                    nc.gpsimd.remote_dma_broadcast(
                        rbuf[:, dt:dt + 1], st_loc[:], rsem, lsem, rdests=rdests)
                nc.gpsimd.trigger_dma(count=None)
                red = spool.tile([128, 1], F32, tag="red")
                ri = nc.vector.tensor_reduce(red[:], rbuf[:],
                                             mybir.AxisListType.X, OP.add)
                # the remote-sem wait is attached AFTER Tile scheduling (the
                # scheduling sim cannot model peer increments and deadlocks)
                rd_ref["ri"] = ri
                rd_ref["rsem"] = rsem
                nc.scalar.copy(st128[0:64, 0:1], red[0:64, :])
                nc.scalar.copy(st128[64:128, 0:1], red[0:64, :])
                nc.scalar.copy(st128[0:64, 1:2], red[64:128, :])
                nc.scalar.copy(st128[64:128, 1:2], red[64:128, :])
            elif USE_AG:
                din = dpool.tile([COUT, 2], F32)
                dout = dpool.tile([NCORES, COUT * 2], F32)
                nc.gpsimd.dma_start(din[:], stat_in[:])
                nc.gpsimd.collective_compute(
                    "AllGather", OP.bypass,
                    replica_groups=[list(range(NCORES))],
                    ins=[din.opt()], outs=[dout.opt()],
                )
                allst = spool.tile([COUT, NCORES * 2], F32, tag="allst")
                nc.scalar.dma_start(
                    allst[:].rearrange("p (r s) -> p r s", r=NCORES, s=2),
                    dout[:].rearrange("r (p s) -> p r s", p=COUT, s=2))
                st64 = spool.tile([COUT, 2], F32, tag="st64")
                av = allst[:].rearrange("p (r s) -> p s r", r=NCORES, s=2)
                nc.vector.tensor_reduce(st64[:], av, mybir.AxisListType.X, OP.add)
                nc.scalar.dma_start(st128[0:64, :], st64[:])
                nc.scalar.dma_start(st128[64:128, :], st64[:])
            else:
                din = dpool.tile([COUT, 2], F32)
                dout = dpool.tile([COUT, 2], F32)
                nc.gpsimd.dma_start(din[:], stat_in[:])
                if USE_AR:
                    nc.gpsimd.collective_compute(
                        "AllReduce", OP.add,
                        replica_groups=[list(range(NCORES))],
                        ins=[din.opt()], outs=[dout.opt()],
                    )
                else:
                    nc.gpsimd.dma_start(dout[:], din[:])
                nc.scalar.dma_start(st128[0:64, :], dout[:])
                nc.scalar.dma_start(st128[64:128, :], dout[:])

            # ---- Z transpose (per batch, DC row folded into bias later) ----
            # zeb[b]: rows 0:64 = Z^T (modes k=1..32, re|im), rows 64:128 = E_b
            Zflat = spool.tile([COUT, 4 * 64], BF16, tag="zflat")  # [b][ri,k>=1]
            nc.scalar.copy(
                Zflat[:].rearrange("p (b ri k) -> p b ri k", b=4, ri=2, k=M - 1),
                Zsb[:].rearrange("p (ri k b) -> p b ri k", ri=2, k=M, b=4)[:, :, :, 1:M])
            for b in range(BLOC):
                tp = psA.tile([CIN, COUT], BF16, tag="ps_tp")
                nc.tensor.transpose(tp[:], Zflat[:, 64 * b:64 * (b + 1)], idb_s[:])
                nc.scalar.copy(zeb[0:64, 64 * b:64 * (b + 1)], tp[:])

            # Zr0 (DC) pair-stacking via partition-shifted engine copies
            zr0p = spool.tile([128, 2], F32, tag="zr0p")
            zr0v = zr0p[:].rearrange("p (j a) -> p j a", j=2, a=1)
            nc.scalar.copy(zr0v[0:64, :, 0],
                           Zsb[:, 0:4].rearrange("p (k b) -> p b k", k=1, b=4)[:, 0:4:2, 0])
            nc.scalar.copy(zr0v[64:128, :, 0],
                           Zsb[:, 0:4].rearrange("p (k b) -> p b k", k=1, b=4)[:, 1:4:2, 0])
            epz = spool.tile([128, 2], F32, tag="epz")
            TT(epz[:], ep_s[:], zr0p[:], OP.add)

            # ---- phase C: single K=128 matmul per tile: [Z;E]^T @ [ABt;xT] ----
            OUT = []
            for j in range(2):
                outj = opool.tile([128, L], BF16, tag=f"out{j}")
                OUT.append(outj)

            NSTEP = 512
            drains = [nc.vector.tensor_copy, nc.scalar.copy]
            di = 0
            for j in range(2):
                b0, b1 = 2 * j, 2 * j + 1
                for n in range(L // NSTEP):
                    ps = psB.tile([128, NSTEP], F32, tag="invres")
                    sl = slice(NSTEP * n, NSTEP * (n + 1))
                    nc.tensor.matmul(ps[0:64, :], zeb[:, 64 * b0:64 * b0 + 64],
                                     xtc[b0][:, sl], start=True, stop=True)
                    nc.tensor.matmul(ps[64:128, :], zeb[:, 64 * b1:64 * b1 + 64],
                                     xtc[b1][:, sl], start=True, stop=True,
                                     tile_position=(0, 64))
                    drains[di % 2](OUT[j][:, sl], ps[:])
                    di += 1

            # ---- BN scale/shift from all-reduced stats ----
            mean = spool.tile([128, 1], F32, tag="mean")
            ex2 = spool.tile([128, 1], F32, tag="ex2")
            var = spool.tile([128, 1], F32, tag="var")
            sv = spool.tile([128, 1], F32, tag="sv")
            sh = spool.tile([128, 1], F32, tag="sh")
            wk = spool.tile([128, 1], F32, tag="wk")
            TS(mean[:], st128[:, 0:1], 1.0 / B, 0.0, OP.mult, OP.add)
            TS(ex2[:], st128[:, 1:2], 1.0 / B, 0.0, OP.mult, OP.add)
            TT(wk[:], mean[:], mean[:], OP.mult)
            TT(var[:], ex2[:], wk[:], OP.subtract)
            TS(var[:], var[:], 1.0, EPS, OP.mult, OP.add)
            nc.scalar.activation(wk[:], var[:], mybir.ActivationFunctionType.Sqrt)
            nc.vector.reciprocal(sv[:], wk[:])
            TT(sv[:], sv[:], bnp_s[:, 0:1], OP.mult)            # s = bn_scale/std
            TT(wk[:], mean[:], sv[:], OP.mult)
            TT(sh[:], bnp_s[:, 1:2], wk[:], OP.subtract)        # shift = bias - mean*s

            bjs = []
            for j in range(2):
                bj = spool.tile([128, 1], F32, tag=f"bj{j}")
                TT(bj[:], epz[:, j:j + 1], sv[:], OP.mult)      # s*(e_b + Zr0)
                TT(bj[:], bj[:], sh[:], OP.add)                 # + shift
                bjs.append(bj)

            # ---- apply BN+ReLU split over ACT(1-pass) / DVE(2-pass, 2x bf16),
            # streaming each applied chunk out on alternating DMA rings ----
            NQ = 1024
            plan = {}
            acts = {(0, 0), (0, 1), (1, 0), (2, 1), (4, 0), (5, 1), (6, 0)}
            wrings = [nc.sync.dma_start, nc.scalar.dma_start]
            for n2 in range(8):
                for j in range(2):
                    q = slice(n2 * NQ, (n2 + 1) * NQ)
                    if (n2, j) in acts:
                        nc.scalar.activation(OUT[j][:, q], OUT[j][:, q],
                                             mybir.ActivationFunctionType.Relu,
                                             bias=bjs[j][:], scale=sv[:])
                    else:
                        TS(OUT[j][:, q], OUT[j][:, q], sv[:], bjs[j][:],
                           OP.mult, OP.add)
                        TS(OUT[j][:, q], OUT[j][:, q], 0.0, 0.0, OP.max, OP.add)
                    od = out_d[2 * j:2 * j + 2].rearrange("a b l -> (a b) l")
                    wrings[(2 * n2 + j) % 2](od[:, q], OUT[j][:, q])

    if rd_ref:
        rd_ref["ri"].wait_op(rd_ref["rsem"], 2 * (NCORES - 1), "sem-ge", check=False)
    nc.compile()
    return nc


_NC_CACHE = {}


def _get_nc():
    if "nc" not in _NC_CACHE:
        _NC_CACHE["nc"] = _build()
    return _NC_CACHE["nc"]


def _host_prep(x, t_emb, spec_w_real, spec_w_imag, dense_re, dense_im,
               conv_kernel, conv_bias, tc_weights, psi_kernel, bn_scale, bn_bias):
    """Build per-core input maps (small tensors precomputed on host)."""
    k = np.arange(M)
    l = np.arange(L)
    ang = 2.0 * np.pi * np.outer(l, k) / L
    CSt = np.concatenate([np.cos(ang) / L, -np.sin(ang) / L], axis=1)   # (L, 66)
    angk = ang[:, 1:]                                # drop DC mode
    ABt = np.concatenate([(2.0 * np.cos(angk)).T,
                          (-2.0 * np.sin(angk)).T], axis=0).astype(np.float32)

    tr = (t_emb @ dense_re).astype(np.float32)      # (B, 33)
    ti = (t_emb @ dense_im).astype(np.float32)
    psi = (t_emb @ psi_kernel).astype(np.float32)
    w_t, b_t = psi[:, :COUT], psi[:, COUT:]
    E = np.einsum("ij,bj,oj->bio", conv_kernel, w_t, tc_weights).astype(np.float32)
    e = ((conv_bias * w_t) @ tc_weights.T + b_t).astype(np.float32)      # (B, 64)

    Wcat = np.concatenate([spec_w_real, spec_w_imag], axis=2)            # (33, 64, 128)
    wm = np.ascontiguousarray(Wcat.transpose(1, 0, 2).reshape(CIN, M * 128)).astype(NP_BF16)
    cstp = np.ascontiguousarray(
        CSt.reshape(NCHUNK, 128, KC).transpose(1, 0, 2)).astype(NP_BF16)  # (128,u,66)
    abt = ABt.astype(NP_BF16)
    idm = np.eye(64, dtype=np.float32)
    bnp = np.stack([np.tile(bn_scale, 2), np.tile(bn_bias, 2)], axis=1).astype(np.float32)

    x16 = x.astype(NP_BF16)
    in_maps = []
    for c in range(NCORES):
        sl = slice(BLOC * c, BLOC * (c + 1))
        xs = x16[sl]                                             # (4, L, 64) bf16
        # per chunk u: [x0 | x1 | cst | x2 | x3] as [128, u, 322]
        xr = xs.reshape(BLOC, NCHUNK, 128, CIN).transpose(0, 2, 1, 3)  # (b,128,u,64)
        xqa = np.empty((128, NCHUNK, CW), NP_BF16)
        xqa[:, :, 0:64] = xr[0]
        xqa[:, :, 64:128] = xr[1]
        xqa[:, :, 128:194] = cstp
        xqa[:, :, 194:258] = xr[2]
        xqa[:, :, 258:322] = xr[3]
        xq = np.ascontiguousarray(xqa.reshape(128, NCHUNK * CW))
        xt = np.ascontiguousarray(xs.transpose(0, 2, 1))         # (4, 64, L) bf16
        trc, tic = tr[sl], ti[sl]                                # (4, 33)
        tmod = np.concatenate([
            trc.T.reshape(-1), tic.T.reshape(-1)                 # [4k+b] each
        ]).astype(np.float32)
        tm = np.broadcast_to(tmod, (COUT, 2 * 4 * M)).copy()
        Ec = E[sl]                                               # (4, 64, 64)
        ec = e[sl]                                               # (4, 64)
        ep = np.stack([
            np.concatenate([ec[0], ec[1]]),
            np.concatenate([ec[2], ec[3]]),
        ], axis=1).astype(np.float32)                            # (128, 2)
        Ecat = np.ascontiguousarray(Ec.transpose(1, 0, 2).reshape(CIN, BLOC * COUT))
        in_maps.append({
            "xq": xq,
            "xt": xt,
            "abt": abt,
            "wm": wm,
            "ebf": Ecat.astype(NP_BF16),
            "ef": Ecat.astype(np.float32),
            "tm": tm,
            "e4": np.ascontiguousarray(ec.T).astype(np.float32),
            "ep": ep,
            "bnp": bnp,
            "idm": idm,
        })
    return in_maps


def kernel(**inputs):
    inputs = {k: np.asarray(v) for k, v in inputs.items()}
    nc = _get_nc()
    in_maps = _host_prep(**inputs)
    res = bass_utils.run_bass_kernel_spmd(
        nc, in_maps, core_ids=list(range(NCORES)),
        trace=bool(int(os.environ.get("KBENCH_TRACE", "0"))),
    )
    out = np.empty((B, L, COUT), np.float32)
    for c in range(NCORES):
        o = res.results[c]["out"].astype(np.float32)     # (4, 64, L)
        out[BLOC * c:BLOC * (c + 1)] = np.ascontiguousarray(o.transpose(0, 2, 1))
    _NC_CACHE["last_results"] = res
    return out


# revision 18
# speedup vs baseline: 1.1292x; 1.0780x over previous
"""Trainium2 Bass kernel for CTUNOBlock1D (spectral conv + time conv + batchnorm + relu).

Strategy (data-parallel over batch, 8 cores, 4 batches/core):
  - rfft uses only 33 modes -> DFT as matmuls against trig tables. x is
    shipped per l-chunk as [x0|x1|cst|x2|x3] so ONE matmul per batch-pair
    per chunk computes both the DFT (X = x^T cst) and the Gram blocks
    (x^T x, for exact BN stats via Parseval) with a 128-wide stationary.
  - mode mixing packs [Wr_k|Wi_k] into one 128-col stationary -> 33 matmuls.
  - BN stats are computed in mode space and AllReduced (64x2 f32) as early
    as possible; the collective overlaps the inverse/residual phase.
  - residual branch folded on host: E_b = K diag(w_t) Wt^T; device computes
    out^T = [Z;E]^T @ [ABt; x^T] per batch (channel-major, K=128 concat).
    PSUM drains are split across Scalar/Vector/Pool engines.
  - BN apply + ReLU is split across Scalar/Vector/Pool with per-chunk
    streaming DMA writes; output is transposed on host.
"""

import os
import numpy as np

import concourse.bass as bass
import concourse.mybir as mybir
import concourse.bacc as bacc
import concourse.tile as tile
from concourse import bass_utils

F32 = mybir.dt.float32
BF16 = mybir.dt.bfloat16
NP_BF16 = mybir.dt.np(BF16)

B, L, CIN, COUT, TEMB = 32, 8192, 64, 64, 256
M = 33            # retained rfft modes
KC = 2 * M        # 66 (real|imag concat)
NCORES = 8
BLOC = B // NCORES   # 4 batches per core
EPS = 1e-5
NCHUNK = L // 128    # 64 l-chunks of 128
CW = 322             # per-chunk xq cols: x0|x1|cst|x2|x3 = 64+64+66+64+64
USE_AR = bool(int(os.environ.get("KBENCH_AR", "1")))
USE_AG = bool(int(os.environ.get("KBENCH_AG", "0")))
USE_RD = bool(int(os.environ.get("KBENCH_RD", "0")))
WARM_CC = bool(int(os.environ.get("KBENCH_WARMCC", "0")))
NXPIECE = 8          # xq DMA split


def _build():
    nc = bacc.Bacc(None, target_bir_lowering=False)

    xq_d = nc.dram_tensor("xq", [128, NCHUNK * CW], BF16, kind="ExternalInput")
    xt_d = nc.dram_tensor("xt", [BLOC, CIN, L], BF16, kind="ExternalInput")
    abt_d = nc.dram_tensor("abt", [CIN, L], BF16, kind="ExternalInput")
    wm_d = nc.dram_tensor("wm", [CIN, M * 128], BF16, kind="ExternalInput")
    ebf_d = nc.dram_tensor("ebf", [CIN, BLOC * COUT], BF16, kind="ExternalInput")
    ef_d = nc.dram_tensor("ef", [CIN, BLOC * COUT], F32, kind="ExternalInput")
    tm_d = nc.dram_tensor("tm", [COUT, 2 * 4 * M], F32, kind="ExternalInput")
    e4_d = nc.dram_tensor("e4", [COUT, BLOC], F32, kind="ExternalInput")
    ep_d = nc.dram_tensor("ep", [128, 2], F32, kind="ExternalInput")
    bnp_d = nc.dram_tensor("bnp", [128, 2], F32, kind="ExternalInput")
    id_d = nc.dram_tensor("idm", [64, 64], F32, kind="ExternalInput")
    out_d = nc.dram_tensor("out", [BLOC, COUT, L], BF16, kind="ExternalOutput")

    rd_ref = {}
    with tile.TileContext(nc) as tc:
        with (
            tc.tile_pool(name="const", bufs=1) as cpool,
            tc.tile_pool(name="xs", bufs=1) as xpool,
            tc.tile_pool(name="xtp", bufs=1) as xtpool,
            tc.tile_pool(name="outb", bufs=1) as opool,
            tc.tile_pool(name="small", bufs=2) as spool,
            tc.tile_pool(name="psA", bufs=1, space=bass.MemorySpace.PSUM) as psA,
            tc.tile_pool(name="psS", bufs=2, space=bass.MemorySpace.PSUM) as psS,
            tc.tile_pool(name="psB", bufs=3, space=bass.MemorySpace.PSUM) as psB,
            tc.tile_pool(name="dram", bufs=1, space=bass.MemorySpace.DRAM) as dpool,
        ):
            dma = nc.sync.dma_start
            TT = nc.vector.tensor_tensor
            TS = nc.vector.tensor_scalar
            gTT = nc.gpsimd.tensor_tensor
            gTS = nc.gpsimd.tensor_scalar
            OP = mybir.AluOpType

            xq = xpool.tile([128, NCHUNK * CW], BF16, tag="xq")
            PW = NCHUNK // NXPIECE * CW
            rings = [nc.sync.dma_start, nc.scalar.dma_start]
            for p in range(NXPIECE):
                rings[p % 2](xq[:, PW * p:PW * (p + 1)], xq_d[:, PW * p:PW * (p + 1)])

            # small constants (own ring position; cheap)
            wm_s = cpool.tile([CIN, M * 128], BF16)
            tm_s = cpool.tile([COUT, 2 * 4 * M], F32)
            e4_s = cpool.tile([COUT, BLOC], F32)
            ep_s = cpool.tile([128, 2], F32)
            bnp_s = cpool.tile([128, 2], F32)
            id_s = cpool.tile([64, 64], F32)
            idb_s = cpool.tile([64, 64], BF16)
            ones_s = cpool.tile([64, 1], F32)
            dma(wm_s[:], wm_d[:])
            dma(tm_s[:], tm_d[:])
            dma(e4_s[:], e4_d[:])
            dma(ep_s[:], ep_d[:])
            dma(bnp_s[:], bnp_d[:])
            dma(id_s[:], id_d[:])
            nc.vector.tensor_copy(idb_s[:], id_s[:])
            nc.vector.memset(ones_s[:], 1.0)

            ebf_s = cpool.tile([CIN, BLOC * COUT], BF16)   # [i, 64b+o]
            ef_s = cpool.tile([CIN, BLOC * COUT], F32)
            zeb = cpool.tile([128, BLOC * COUT], BF16)     # [0:64]=Z^T, [64:128]=E
            dma(ebf_s[:], ebf_d[:])
            dma(ef_s[:], ef_d[:])
            dma(zeb[64:128, :], ebf_d[:])

            # early dummy collective: absorbs the cross-core launch barrier
            # and warms the CC rings while compute proceeds
            if WARM_CC and (USE_AR or USE_AG) and not USE_RD:
                wtin = dpool.tile([2, 2], F32)
                wtout = dpool.tile([2, 2], F32)
                wsrc = spool.tile([2, 2], F32, tag="wsrc")
                nc.vector.memset(wsrc[:], 0.0)
                nc.gpsimd.dma_start(wtin[:], wsrc[:])
                nc.gpsimd.collective_compute(
                    "AllReduce", mybir.AluOpType.add,
                    replica_groups=[list(range(NCORES))],
                    ins=[wtin.opt()], outs=[wtout.opt()],
                )

            # early dummy Sqrt to pre-load the ACT table set, and a dummy
            # gpsimd op to pre-load the Pool ext-isa library
            warm = spool.tile([1, 1], F32)
            nc.vector.memset(warm[:], 1.0)
            nc.scalar.activation(warm[:], warm[:], mybir.ActivationFunctionType.Sqrt)
            warm2 = spool.tile([1, 1], F32, tag="warm2")
            nc.gpsimd.tensor_tensor(warm2[:], warm[:], warm[:], mybir.AluOpType.add)

            # bulk phase-C inputs, gated behind the critical xq loads: the
            # dummy read of the last xq piece makes the sync ring wait before
            # issuing these transfers (keeps HBM bandwidth on the fwd path).
            gate = spool.tile([1, 2], BF16, tag="gate")
            dma(gate[:], xq[0:1, NCHUNK * CW - 2:NCHUNK * CW])
            gate2 = spool.tile([1, 2], BF16, tag="gate2")
            nc.scalar.dma_start(gate2[:], xq[0:1, PW * 7 - 2:PW * 7])
            xtc = []
            for b in range(BLOC):
                xt = xtpool.tile([128, L], BF16, tag=f"xtc{b}")
                xtc.append(xt)
            nc.scalar.dma_start(xtc[0][0:64, :], abt_d[:])
            dma(xtc[0][64:128, :], xt_d[0])
            nc.scalar.dma_start(xtc[1][64:128, :], xt_d[1])
            dma(xtc[2][64:128, :], xt_d[2])
            nc.scalar.dma_start(xtc[3][64:128, :], xt_d[3])
            # replicate the ABt table into the other xtc tops (sync ring,
            # which is idle once xt b0/b2 have landed)
            dma(xtc[1][0:64, :], xtc[0][0:64, :])
            dma(xtc[2][0:64, :], xtc[0][0:64, :])
            dma(xtc[3][0:64, :], xtc[1][0:64, :])

            # ---- phase A: one matmul per batch-pair per chunk ----
            # pair0: [x0|x1]^T @ [x0|x1|cst] -> [G00 G01; G10 G11 | X0; X1]
            # pair1: [x2|x3]^T @ [cst|x2|x3] -> [X2; X3 | G22 .. ; .. G33]
            pA0 = psA.tile([128, 194], F32, tag="pA0")
            pA1 = psA.tile([128, 194], F32, tag="pA1")
            for u in range(NCHUNK):
                o = CW * u
                nc.tensor.matmul(pA0[:], xq[:, o:o + 128], xq[:, o:o + 194],
                                 start=(u == 0), stop=(u == NCHUNK - 1))
                nc.tensor.matmul(pA1[:], xq[:, o + 194:o + 322], xq[:, o + 128:o + 322],
                                 start=(u == 0), stop=(u == NCHUNK - 1))

            # ---- copies out of phase-A PSUM (split across engines) ----
            Xsb = cpool.tile([CIN, BLOC * KC], BF16)    # [c, 66b+(ri,k)]
            Gsb = cpool.tile([CIN, BLOC * CIN], BF16)   # [c, 64b+c']
            nc.scalar.copy(Xsb[:, 0:66], pA0[0:64, 128:194])
            nc.vector.tensor_copy(Xsb[:, 66:132], pA0[64:128, 128:194])
            nc.scalar.copy(Xsb[:, 132:198], pA1[0:64, 0:66])
            nc.scalar.copy(Xsb[:, 198:264], pA1[64:128, 0:66])
            nc.vector.tensor_copy(Gsb[:, 0:64], pA0[0:64, 0:64])
            nc.vector.tensor_copy(Gsb[:, 64:128], pA0[64:128, 64:128])
            nc.scalar.copy(Gsb[:, 128:192], pA1[0:64, 66:130])
            nc.vector.tensor_copy(Gsb[:, 192:256], pA1[64:128, 130:194])

            # ---- phase B: mode mixing, one matmul per mode ----
            # P[0:64,8k+(j,b)] = Wr_k^T @ [Xr|Xi]; P[64:128,...] = Wi_k^T @ ...
            Pp = psS.tile([128, M * 8], F32, tag="ps_s")
            Xr4 = Xsb[:].rearrange("p (b j k) -> p j b k", b=BLOC, j=2, k=M)
            for k in range(M):
                nc.tensor.matmul(Pp[:, 8 * k:8 * (k + 1)],
                                 wm_s[:, 128 * k:128 * (k + 1)],
                                 Xr4[:, :, :, k], start=True, stop=True)

            # Gm / m1p matmuls (independent of P; share the PE queue)
            gmp = psS.tile([COUT, BLOC * KC], F32, tag="ps_s")
            m1p = psS.tile([CIN, BLOC * COUT], F32, tag="ps_s")
            for b in range(BLOC):
                nc.tensor.matmul(gmp[:, KC * b:KC * (b + 1)],
                                 ebf_s[:, 64 * b:64 * (b + 1)],
                                 Xsb[:, KC * b:KC * (b + 1)], start=True, stop=True)
                nc.tensor.matmul(m1p[:, 64 * b:64 * (b + 1)],
                                 Gsb[:, 64 * b:64 * (b + 1)],
                                 ebf_s[:, 64 * b:64 * (b + 1)], start=True, stop=True)

            Psb = spool.tile([COUT, 2 * M * 8], F32, tag="psb")
            nc.scalar.copy(Psb[:, 0:M * 8], Pp[0:64, :])
            nc.vector.tensor_copy(Psb[:, M * 8:2 * M * 8], Pp[64:128, :])

            Gm = cpool.tile([COUT, BLOC * KC], F32)     # [o, 66b+33ri+k]
            nc.scalar.copy(Gm[:], gmp[:])
            em = spool.tile([CIN, BLOC * COUT], F32, tag="em")
            TT(em[:], m1p[:], ef_s[:], OP.mult)
            qp = psS.tile([COUT, BLOC], F32, tag="ps_s")
            for b in range(BLOC):
                nc.tensor.matmul(qp[:, b:b + 1], em[:, 64 * b:64 * (b + 1)],
                                 ones_s[:], start=True, stop=True)

            Yr = spool.tile([COUT, 4 * M], F32, tag="yr")   # [(k,b)] = 4k+b
            Yi = spool.tile([COUT, 4 * M], F32, tag="yi")
            Pk1 = Psb[:, 0:M * 8].rearrange("p (k x) -> p k x", k=M, x=8)
            Pk2 = Psb[:, M * 8:2 * M * 8].rearrange("p (k x) -> p k x", k=M, x=8)
            Yrv = Yr[:].rearrange("p (k b) -> p k b", k=M, b=4)
            Yiv = Yi[:].rearrange("p (k b) -> p k b", k=M, b=4)
            TT(Yrv, Pk1[:, :, 0:4], Pk2[:, :, 4:8], OP.subtract)
            gTT(Yiv, Pk2[:, :, 0:4], Pk1[:, :, 4:8], OP.add)
            # Yr used by DVE (t1,t4), Yi by Pool (t2,t3) -- minimal cross-hops

            Zsb = cpool.tile([COUT, 2 * 4 * M], F32)  # [(ri,k,b)] = 132ri+4k+b
            t1 = spool.tile([COUT, 4 * M], F32, tag="t1")
            t2 = spool.tile([COUT, 4 * M], F32, tag="t2")
            t3 = spool.tile([COUT, 4 * M], F32, tag="t3")
            t4 = spool.tile([COUT, 4 * M], F32, tag="t4")
            ntm = 4 * M
            TT(t1[:], Yr[:], tm_s[:, 0:ntm], OP.mult)
            TT(t4[:], Yr[:], tm_s[:, ntm:2 * ntm], OP.mult)
            gTT(t2[:], Yi[:], tm_s[:, ntm:2 * ntm], OP.mult)
            gTT(t3[:], Yi[:], tm_s[:, 0:ntm], OP.mult)
            TT(Zsb[:, 0:ntm], t1[:], t2[:], OP.subtract)
            TT(Zsb[:, ntm:2 * ntm], t3[:], t4[:], OP.add)

            # ---- stats in mode space (batched over the 4 batches) ----
            q4 = spool.tile([COUT, BLOC], F32, tag="q4")
            A12 = spool.tile([COUT, BLOC], F32, tag="a12")
            nc.vector.tensor_copy(q4[:], qp[:])
            # A12 = sum_k>=1 Zr*(Zr+2Gr) + Zi*(Zi+2Gi)  (= A1 + 2*A2)
            Zall = Zsb[:].rearrange("p (ri k b) -> p b ri k", ri=2, k=M, b=4)[:, :, :, 1:M]
            Gall = Gm[:].rearrange("p (b ri k) -> p b ri k", b=BLOC, ri=2, k=M)[:, :, :, 1:M]
            w256a = spool.tile([COUT, BLOC * 64], F32, tag="w256a")
            w256b = spool.tile([COUT, BLOC * 64], F32, tag="w256b")
            wa = w256a[:].rearrange("p (b ri k) -> p b ri k", b=4, ri=2, k=M - 1)
            wb = w256b[:].rearrange("p (b ri k) -> p b ri k", b=4, ri=2, k=M - 1)
            gTS(wa, Gall, 2.0, 0.0, OP.mult, OP.add)
            TT(wb, Zall, wa, OP.add)
            TT(wa, Zall, wb, OP.mult)
            nc.vector.tensor_reduce(
                A12[:], w256a[:].rearrange("p (b k) -> p b k", b=BLOC, k=64),
                mybir.AxisListType.X, OP.add)

            # vectorized S1/S2 assembly over the 4 batches
            Zr04 = Zsb[:, 0:4]                                  # Zr[k=0] per b
            u4 = Gm[:].rearrange("p (b x) -> p b x", b=BLOC, x=KC)[:, :, 0]
            v4 = spool.tile([COUT, BLOC], F32, tag="v4")
            s2c = spool.tile([COUT, BLOC], F32, tag="s2c")
            w1 = spool.tile([COUT, BLOC], F32, tag="w1")
            w2 = spool.tile([COUT, BLOC], F32, tag="w2")
            TT(v4[:], Zr04, u4, OP.add)
            TT(v4[:], v4[:], e4_s[:], OP.add)                   # v = Zr0+u+e
            gTT(w2[:], Zr04, u4, OP.mult)                       # Zr0*u
            TT(s2c[:], Zr04, Zr04, OP.mult)                     # Zr0^2
            TS(w1[:], A12[:], 2.0, 0.0, OP.mult, OP.add)
            TT(s2c[:], s2c[:], w1[:], OP.add)
            TS(w1[:], q4[:], 1.0 / L, 0.0, OP.mult, OP.add)
            TT(s2c[:], s2c[:], w1[:], OP.add)
            TT(w1[:], e4_s[:], v4[:], OP.mult)
            TT(w1[:], w1[:], w2[:], OP.add)                     # e*v + Zr0*u
            TS(w1[:], w1[:], 2.0, 0.0, OP.mult, OP.add)
            TT(s2c[:], s2c[:], w1[:], OP.add)
            TT(w1[:], e4_s[:], e4_s[:], OP.mult)
            TT(s2c[:], s2c[:], w1[:], OP.subtract)

            stat_in = spool.tile([COUT, 2], F32, tag="stin")
            nc.vector.tensor_reduce(stat_in[:, 0:1], v4[:], mybir.AxisListType.X, OP.add)
            nc.vector.tensor_reduce(stat_in[:, 1:2], s2c[:], mybir.AxisListType.X, OP.add)

            # ---- cross-core reduction of (64,2) stats ----
            st128 = spool.tile([128, 2], F32, tag="st128")
            if USE_RD:
                # P2P stats exchange over remote SBUF-to-SBUF DMA: each core
                # broadcasts its 512B stats to the 7 peers (slot j written by
                # peer me^j), then reduces locally once 7x2 sem bumps land.
                rsem = nc.alloc_semaphore(name="rd_rsem")
                lsem = nc.alloc_semaphore(name="rd_lsem")
                st_loc = cpool.tile([128, 1], F32)   # v on 0:64, s2 on 64:128
                nc.scalar.copy(st_loc[0:64, :], stat_in[:, 0:1])
                nc.scalar.copy(st_loc[64:128, :], stat_in[:, 1:2])
                rbuf = cpool.tile([128, 8], F32)
                nc.vector.tensor_copy(rbuf[:, 0:1], st_loc[:])   # self slot 0
                for dt in range(1, NCORES):
                    rdests = [None] * 8
                    rdests[dt] = (0, dt)
                    nc.gpsimd.remote_dma_broadcast(
                        rbuf[:, dt:dt + 1], st_loc[:], rsem, lsem, rdests=rdests)
                nc.gpsimd.trigger_dma(count=None)
                red = spool.tile([128, 1], F32, tag="red")
                ri = nc.vector.tensor_reduce(red[:], rbuf[:],
                                             mybir.AxisListType.X, OP.add)
                # the remote-sem wait is attached AFTER Tile scheduling (the
                # scheduling sim cannot model peer increments and deadlocks)
                rd_ref["ri"] = ri
                rd_ref["rsem"] = rsem
                nc.scalar.copy(st128[0:64, 0:1], red[0:64, :])
                nc.scalar.copy(st128[64:128, 0:1], red[0:64, :])
                nc.scalar.copy(st128[0:64, 1:2], red[64:128, :])
                nc.scalar.copy(st128[64:128, 1:2], red[64:128, :])
            elif USE_AG:
                din = dpool.tile([COUT, 2], F32)
                dout = dpool.tile([NCORES, COUT * 2], F32)
                nc.gpsimd.dma_start(din[:], stat_in[:])
                nc.gpsimd.collective_compute(
                    "AllGather", OP.bypass,
                    replica_groups=[list(range(NCORES))],
                    ins=[din.opt()], outs=[dout.opt()],
                )
                allst = spool.tile([COUT, NCORES * 2], F32, tag="allst")
                nc.scalar.dma_start(
                    allst[:].rearrange("p (r s) -> p r s", r=NCORES, s=2),
                    dout[:].rearrange("r (p s) -> p r s", p=COUT, s=2))
                st64 = spool.tile([COUT, 2], F32, tag="st64")
                av = allst[:].rearrange("p (r s) -> p s r", r=NCORES, s=2)
                nc.vector.tensor_reduce(st64[:], av, mybir.AxisListType.X, OP.add)
                nc.scalar.dma_start(st128[0:64, :], st64[:])
                nc.scalar.dma_start(st128[64:128, :], st64[:])
            else:
                din = dpool.tile([COUT, 2], F32)
                dout = dpool.tile([COUT, 2], F32)
                nc.gpsimd.dma_start(din[:], stat_in[:])
                if USE_AR:
                    nc.gpsimd.collective_compute(
                        "AllReduce", OP.add,
                        replica_groups=[list(range(NCORES))],
                        ins=[din.opt()], outs=[dout.opt()],
                    )
                else:
                    nc.gpsimd.dma_start(dout[:], din[:])
                nc.scalar.dma_start(st128[0:64, :], dout[:])
                nc.scalar.dma_start(st128[64:128, :], dout[:])

            # ---- Z transpose (per batch, DC row folded into bias later) ----
            # zeb[b]: rows 0:64 = Z^T (modes k=1..32, re|im), rows 64:128 = E_b
            Zflat = spool.tile([COUT, 4 * 64], BF16, tag="zflat")  # [b][ri,k>=1]
            nc.scalar.copy(
                Zflat[:].rearrange("p (b ri k) -> p b ri k", b=4, ri=2, k=M - 1),
                Zsb[:].rearrange("p (ri k b) -> p b ri k", ri=2, k=M, b=4)[:, :, :, 1:M])
            for b in range(BLOC):
                tp = psA.tile([CIN, COUT], BF16, tag="ps_tp")
                nc.tensor.transpose(tp[:], Zflat[:, 64 * b:64 * (b + 1)], idb_s[:])
                nc.scalar.copy(zeb[0:64, 64 * b:64 * (b + 1)], tp[:])

            # Zr0 (DC) pair-stacking via partition-shifted engine copies
            zr0p = spool.tile([128, 2], F32, tag="zr0p")
            zr0v = zr0p[:].rearrange("p (j a) -> p j a", j=2, a=1)
            nc.scalar.copy(zr0v[0:64, :, 0],
                           Zsb[:, 0:4].rearrange("p (k b) -> p b k", k=1, b=4)[:, 0:4:2, 0])
            nc.scalar.copy(zr0v[64:128, :, 0],
                           Zsb[:, 0:4].rearrange("p (k b) -> p b k", k=1, b=4)[:, 1:4:2, 0])
            epz = spool.tile([128, 2], F32, tag="epz")
            TT(epz[:], ep_s[:], zr0p[:], OP.add)

            # ---- phase C: single K=128 matmul per tile: [Z;E]^T @ [ABt;xT] ----
            OUT = []
            for j in range(2):
                outj = opool.tile([128, L], BF16, tag=f"out{j}")
                OUT.append(outj)

            NSTEP = 512
            drains = [nc.vector.tensor_copy, nc.scalar.copy]
            di = 0
            for j in range(2):
                b0, b1 = 2 * j, 2 * j + 1
                for n in range(L // NSTEP):
                    ps = psB.tile([128, NSTEP], F32, tag="invres")
                    sl = slice(NSTEP * n, NSTEP * (n + 1))
                    nc.tensor.matmul(ps[0:64, :], zeb[:, 64 * b0:64 * b0 + 64],
                                     xtc[b0][:, sl], start=True, stop=True)
                    nc.tensor.matmul(ps[64:128, :], zeb[:, 64 * b1:64 * b1 + 64],
                                     xtc[b1][:, sl], start=True, stop=True,
                                     tile_position=(0, 64))
                    drains[di % 2](OUT[j][:, sl], ps[:])
                    di += 1

            # ---- BN scale/shift from all-reduced stats ----
            mean = spool.tile([128, 1], F32, tag="mean")
            ex2 = spool.tile([128, 1], F32, tag="ex2")
            var = spool.tile([128, 1], F32, tag="var")
            sv = spool.tile([128, 1], F32, tag="sv")
            sh = spool.tile([128, 1], F32, tag="sh")
            wk = spool.tile([128, 1], F32, tag="wk")
            TS(mean[:], st128[:, 0:1], 1.0 / B, 0.0, OP.mult, OP.add)
            TS(ex2[:], st128[:, 1:2], 1.0 / B, 0.0, OP.mult, OP.add)
            TT(wk[:], mean[:], mean[:], OP.mult)
            TT(var[:], ex2[:], wk[:], OP.subtract)
            TS(var[:], var[:], 1.0, EPS, OP.mult, OP.add)
            nc.scalar.activation(wk[:], var[:], mybir.ActivationFunctionType.Sqrt)
            nc.vector.reciprocal(sv[:], wk[:])
            TT(sv[:], sv[:], bnp_s[:, 0:1], OP.mult)            # s = bn_scale/std
            TT(wk[:], mean[:], sv[:], OP.mult)
            TT(sh[:], bnp_s[:, 1:2], wk[:], OP.subtract)        # shift = bias - mean*s

            bjs = []
            for j in range(2):
                bj = spool.tile([128, 1], F32, tag=f"bj{j}")
                TT(bj[:], epz[:, j:j + 1], sv[:], OP.mult)      # s*(e_b + Zr0)
                TT(bj[:], bj[:], sh[:], OP.add)                 # + shift
                bjs.append(bj)

            # ---- apply BN+ReLU split over ACT(1-pass) / DVE(2-pass, 2x bf16),
            # streaming each applied chunk out on alternating DMA rings ----
            NQ = 1024
            plan = {}
            acts = {(0, 0), (0, 1), (1, 0), (2, 1), (4, 0), (5, 1), (6, 0)}
            wrings = [nc.sync.dma_start, nc.scalar.dma_start, nc.gpsimd.dma_start]
            for n2 in range(8):
                for j in range(2):
                    q = slice(n2 * NQ, (n2 + 1) * NQ)
                    if (n2, j) in acts:
                        nc.scalar.activation(OUT[j][:, q], OUT[j][:, q],
                                             mybir.ActivationFunctionType.Relu,
                                             bias=bjs[j][:], scale=sv[:])
                    else:
                        TS(OUT[j][:, q], OUT[j][:, q], sv[:], bjs[j][:],
                           OP.mult, OP.add)
                        TS(OUT[j][:, q], OUT[j][:, q], 0.0, 0.0, OP.max, OP.add)
                    od = out_d[2 * j:2 * j + 2].rearrange("a b l -> (a b) l")
                    wrings[(2 * n2 + j) % 3](od[:, q], OUT[j][:, q])

    if rd_ref:
        rd_ref["ri"].wait_op(rd_ref["rsem"], 2 * (NCORES - 1), "sem-ge", check=False)
    nc.compile()
    return nc


_NC_CACHE = {}


def _get_nc():
    if "nc" not in _NC_CACHE:
        _NC_CACHE["nc"] = _build()
    return _NC_CACHE["nc"]


def _host_prep(x, t_emb, spec_w_real, spec_w_imag, dense_re, dense_im,
               conv_kernel, conv_bias, tc_weights, psi_kernel, bn_scale, bn_bias):
    """Build per-core input maps (small tensors precomputed on host)."""
    k = np.arange(M)
    l = np.arange(L)
    ang = 2.0 * np.pi * np.outer(l, k) / L
    CSt = np.concatenate([np.cos(ang) / L, -np.sin(ang) / L], axis=1)   # (L, 66)
    angk = ang[:, 1:]                                # drop DC mode
    ABt = np.concatenate([(2.0 * np.cos(angk)).T,
                          (-2.0 * np.sin(angk)).T], axis=0).astype(np.float32)

    tr = (t_emb @ dense_re).astype(np.float32)      # (B, 33)
    ti = (t_emb @ dense_im).astype(np.float32)
    psi = (t_emb @ psi_kernel).astype(np.float32)
    w_t, b_t = psi[:, :COUT], psi[:, COUT:]
    E = np.einsum("ij,bj,oj->bio", conv_kernel, w_t, tc_weights).astype(np.float32)
    e = ((conv_bias * w_t) @ tc_weights.T + b_t).astype(np.float32)      # (B, 64)

    Wcat = np.concatenate([spec_w_real, spec_w_imag], axis=2)            # (33, 64, 128)
    wm = np.ascontiguousarray(Wcat.transpose(1, 0, 2).reshape(CIN, M * 128)).astype(NP_BF16)
    cstp = np.ascontiguousarray(
        CSt.reshape(NCHUNK, 128, KC).transpose(1, 0, 2)).astype(NP_BF16)  # (128,u,66)
    abt = ABt.astype(NP_BF16)
    idm = np.eye(64, dtype=np.float32)
    bnp = np.stack([np.tile(bn_scale, 2), np.tile(bn_bias, 2)], axis=1).astype(np.float32)

    x16 = x.astype(NP_BF16)
    in_maps = []
    for c in range(NCORES):
        sl = slice(BLOC * c, BLOC * (c + 1))
        xs = x16[sl]                                             # (4, L, 64) bf16
        # per chunk u: [x0 | x1 | cst | x2 | x3] as [128, u, 322]
        xr = xs.reshape(BLOC, NCHUNK, 128, CIN).transpose(0, 2, 1, 3)  # (b,128,u,64)
        xqa = np.empty((128, NCHUNK, CW), NP_BF16)
        xqa[:, :, 0:64] = xr[0]
        xqa[:, :, 64:128] = xr[1]
        xqa[:, :, 128:194] = cstp
        xqa[:, :, 194:258] = xr[2]
        xqa[:, :, 258:322] = xr[3]
        xq = np.ascontiguousarray(xqa.reshape(128, NCHUNK * CW))
        xt = np.ascontiguousarray(xs.transpose(0, 2, 1))         # (4, 64, L) bf16
        trc, tic = tr[sl], ti[sl]                                # (4, 33)
        tmod = np.concatenate([
            trc.T.reshape(-1), tic.T.reshape(-1)                 # [4k+b] each
        ]).astype(np.float32)
        tm = np.broadcast_to(tmod, (COUT, 2 * 4 * M)).copy()
        Ec = E[sl]                                               # (4, 64, 64)
        ec = e[sl]                                               # (4, 64)
        ep = np.stack([
            np.concatenate([ec[0], ec[1]]),
            np.concatenate([ec[2], ec[3]]),
        ], axis=1).astype(np.float32)                            # (128, 2)
        Ecat = np.ascontiguousarray(Ec.transpose(1, 0, 2).reshape(CIN, BLOC * COUT))
        in_maps.append({
            "xq": xq,
            "xt": xt,
            "abt": abt,
            "wm": wm,
            "ebf": Ecat.astype(NP_BF16),
            "ef": Ecat.astype(np.float32),
            "tm": tm,
            "e4": np.ascontiguousarray(ec.T).astype(np.float32),
            "ep": ep,
            "bnp": bnp,
            "idm": idm,
        })
    return in_maps


def kernel(**inputs):
    inputs = {k: np.asarray(v) for k, v in inputs.items()}
    nc = _get_nc()
    in_maps = _host_prep(**inputs)
    res = bass_utils.run_bass_kernel_spmd(
        nc, in_maps, core_ids=list(range(NCORES)),
        trace=bool(int(os.environ.get("KBENCH_TRACE", "0"))),
    )
    out = np.empty((B, L, COUT), np.float32)
    for c in range(NCORES):
        o = res.results[c]["out"].astype(np.float32)     # (4, 64, L)
        out[BLOC * c:BLOC * (c + 1)] = np.ascontiguousarray(o.transpose(0, 2, 1))
    _NC_CACHE["last_results"] = res
    return out
